# revision 26
# baseline (speedup 1.0000x reference)
"""Trainium2 Bass kernel for nn_ARVideoPatchTransformer_80436147519663.

Distribution: data-parallel over batch (B=32 -> 4 samples/core x 8 cores),
no collectives. Each core runs the full transformer on its shard.

On-chip dataflow (per core):
- Residual x is feature-major [D(4x128 partition-tiles), 2048 tokens], fp32.
- All matmuls run in float32r (full PE rate at N>=256, ~14-bit mantissa),
  accumulating fp32 in PSUM.
- QKV emits q,k feature-major (scores need hd on partitions) and v
  token-major (AV matmul wants l_k on partitions). No PE transposes anywhere.
- RoPE pair-swap = PE permutation matmul; rope mul/add on DVE (q) and
  GPSIMD (k) to balance engines.
- Softmax in scoresT orientation [l_k, l_q]: block-causal mask realized by
  restricting matmul column ranges + one 64x64 corner memset per tile;
  per-head q-rms (with 1/sqrt(HD) folded) is produced pre-broadcast by a
  block-ones stationary matmul; k-rms lands token-major via stationary-k^2
  matmuls and is folded into the exp() activation scale. Softmax denominators
  come from a 64-col ones matmul (pre-broadcast), one reciprocal per 2 heads.
"""
import math
import numpy as np

import concourse.bass as bass
from concourse import bacc
import concourse.mybir as mybir
import concourse.tile as tile
from concourse.bass_utils import run_bass_kernel_spmd

F32 = mybir.dt.float32
F32R = mybir.dt.float32r
AF = mybir.ActivationFunctionType
ALU = mybir.AluOpType

B, T, C, RES, P = 32, 8, 3, 64, 8
NP = (RES // P) ** 2          # 64
L = T * NP                    # 512
PD = C * P * P                # 192
D, NH, NL = 512, 8, 8
HD = D // NH                  # 64
RD = HD // 2                  # 32
INNER = 1364
IH = INNER // 2               # 682
IHP = 768                     # padded half (6*128)
MAXT = T + 1
EPS = 1e-6
NCORES = 8
BL = B // NCORES              # 4
NTOK = BL * L                 # 2048
DT = D // 128                 # 4


# ---------------------------------------------------------------- host prep

def _sin_cos(rotary_dim, max_len, base=10000.0):
    inv = 1.0 / (base ** (np.arange(0, rotary_dim, 2, dtype=np.float32) / rotary_dim))
    ang = np.outer(np.arange(max_len, dtype=np.float32), inv)
    ang = np.repeat(ang, 2, axis=-1)
    return np.cos(ang).astype(np.float32), np.sin(ang).astype(np.float32)


def _build_rope_tables(q_scale, k_scale):
    """[128(2 heads' feats), 512(l)] C and S tables with scale folded."""
    t_cos, t_sin = _sin_cos(RD, MAXT)
    s_cos, s_sin = _sin_cos(RD, NP)
    l = np.arange(L)
    cos_full = np.concatenate([t_cos[l // NP], s_cos[l % NP]], axis=1)  # [512, 64]
    sin_full = np.concatenate([t_sin[l // NP], s_sin[l % NP]], axis=1)

    def fold(s):
        s = np.asarray(s, np.float32)
        Cm = cos_full * s[None, :]
        Sm = np.empty_like(sin_full)
        Sm[:, 0::2] = -sin_full[:, 0::2] * s[None, 1::2]
        Sm[:, 1::2] = sin_full[:, 1::2] * s[None, 0::2]
        return np.tile(Cm.T, (2, 1)).copy(), np.tile(Sm.T, (2, 1)).copy()  # [128, 512]

    return fold(q_scale), fold(k_scale)


def _patchify(frames):
    b, t, c, hh, ww = frames.shape
    h = hh // P
    x = frames.reshape(b * t, c, h, P, h, P)
    x = x.transpose(0, 2, 4, 1, 3, 5)
    return x.reshape(b, t * h * h, c * P * P)


def _unpatchify(tokens):
    b, l, _ = tokens.shape
    h = RES // P
    t = l // (h * h)
    x = tokens.reshape(b * t, h, h, C, P, P)
    x = x.transpose(0, 3, 1, 4, 2, 5)
    return x.reshape(b, t, C, h * P, h * P)


def host_prep(params):
    def np32(a):
        return np.ascontiguousarray(np.asarray(a, dtype=np.float32))

    out = {}
    wpe = np.zeros((256, 512), np.float32)
    wpe[:192] = np32(params["patch_embed"])
    out["wpe"] = wpe
    out["enorm"] = np32(params["embed_norm"]).reshape(DT, 128).T.copy()  # [128, 4]

    qs0 = np32(params["layers"][0]["q_scale"])
    ks0 = np32(params["layers"][0]["k_scale"])
    same_tables = all(
        np.array_equal(np32(lp["q_scale"]), qs0) and np.array_equal(np32(lp["k_scale"]), ks0)
        for lp in params["layers"])
    out["same_tables"] = same_tables

    for i, lp in enumerate(params["layers"]):
        wqkv = np32(lp["qkv"]) * np32(lp["norm1"])[:, None]
        out[f"wqk_{i}"] = np.ascontiguousarray(wqkv[:, :1024])
        out[f"wv_{i}"] = np.ascontiguousarray(wqkv[:, 1024:])
        out[f"wout_{i}"] = np32(lp["out"])
        gate = np32(lp["gate"]) * np32(lp["norm2"])[:, None]
        up = np32(lp["up"]) * np32(lp["norm2"])[:, None]
        wgu = np.zeros((6, 512, 512), np.float32)
        for f in range(6):
            lo, hi = f * 128, min(f * 128 + 128, IH)
            if hi > lo:
                for j, src in enumerate((gate[:, :IH], gate[:, IH:], up[:, :IH], up[:, IH:])):
                    wgu[f, :, j * 128:j * 128 + hi - lo] = src[:, lo:hi]
        out[f"wgu_{i}"] = wgu
        wdown = np.zeros((IHP, 512), np.float32)
        wdown[:IH] = np32(lp["down"])
        out[f"wdown_{i}"] = wdown
        if i == 0 or not same_tables:
            (cq, sq), (ck, sk) = _build_rope_tables(lp["q_scale"], lp["k_scale"])
            out[f"ropeq_{i}"] = np.concatenate([cq, sq], axis=1)   # [128, 1024]
            out[f"ropek_{i}"] = np.concatenate([ck, sk], axis=1)

    whead = np.zeros((512, 256), np.float32)
    whead[:, :192] = np32(params["out_norm"])[:, None] * np32(params["head"])
    out["whead"] = whead

    consts = np.zeros((128, 2), np.float32)
    consts[:, :] = 1.0
    out["consts"] = consts
    out["allones"] = np.ones((128, 128), np.float32)
    oneblocks = np.zeros((128, 192), np.float32)
    oneblocks[:, 0:64] = 1.0
    oneblocks[:, 128:192] = 1.0
    out["oneblocks"] = oneblocks
    hsel = np.zeros((128, 2), np.float32)
    hsel[:64, 0] = 1.0
    hsel[64:, 1] = 1.0
    out["hsel"] = hsel
    biases = np.zeros((128, 2), np.float32)
    biases[:, 0] = EPS
    biases[:, 1] = HD * EPS
    out["biases"] = biases
    psw = np.zeros((128, 128), np.float32)
    for i in range(64):
        psw[2 * i, 2 * i + 1] = 1.0
        psw[2 * i + 1, 2 * i] = 1.0
    out["pswap"] = psw
    bones = np.zeros((128, 128), np.float32)
    bones[:64, :64] = 1.0
    bones[64:, 64:] = 1.0
    out["bones"] = bones
    return out


# ---------------------------------------------------------------- bass build

def _patch_act_tables():
    """Restrict the act-table chooser to {natural_log_exp_and_others, silu_and_others}
    so the greedy first-match picker stops thrashing between per-function sets.
    Indices are preserved (walrus maps set-id -> table by position)."""
    import concourse.hw_specs as hw_specs
    if getattr(hw_specs, "_act_tables_patched", False):
        return
    orig = hw_specs.get_activation_tables

    def patched(module_arch):
        tabs = orig(module_arch)
        keep = {"natural_log_exp_and_others", "silu_and_others"}
        return {k: (v if k in keep else set()) for k, v in tabs.items()}

    hw_specs.get_activation_tables = patched
    bacc.get_activation_tables = patched
    hw_specs._act_tables_patched = True


def build_nc(same_tables=True):
    _patch_act_tables()
    nc = bacc.Bacc()

    x0_d = nc.dram_tensor("x0", [256, NTOK], F32R, kind="ExternalInput")
    wpe_d = nc.dram_tensor("wpe", [256, 512], F32R, kind="ExternalInput")
    enorm_d = nc.dram_tensor("enorm", [128, DT], F32, kind="ExternalInput")
    whead_d = nc.dram_tensor("whead", [512, 256], F32R, kind="ExternalInput")
    consts_d = nc.dram_tensor("consts", [128, 2], F32R, kind="ExternalInput")
    allones_d = nc.dram_tensor("allones", [128, 128], F32R, kind="ExternalInput")
    oneblocks_d = nc.dram_tensor("oneblocks", [128, 192], F32R, kind="ExternalInput")
    pswap_d = nc.dram_tensor("pswap", [128, 128], F32R, kind="ExternalInput")
    bones_d = nc.dram_tensor("bones", [128, 128], F32R, kind="ExternalInput")
    biases_d = nc.dram_tensor("biases", [128, 2], F32, kind="ExternalInput")
    hsel_d = nc.dram_tensor("hsel", [128, 2], F32R, kind="ExternalInput")
    wqk_d, wv_d, wout_d, wgu_d, wdown_d, ropeq_d, ropek_d = [], [], [], [], [], [], []
    ntab = 1 if same_tables else NL
    for i in range(NL):
        wqk_d.append(nc.dram_tensor(f"wqk_{i}", [512, 1024], F32R, kind="ExternalInput"))
        wv_d.append(nc.dram_tensor(f"wv_{i}", [512, 512], F32R, kind="ExternalInput"))
        wout_d.append(nc.dram_tensor(f"wout_{i}", [512, 512], F32R, kind="ExternalInput"))
        wgu_d.append(nc.dram_tensor(f"wgu_{i}", [6, 512, 512], F32R, kind="ExternalInput"))
        wdown_d.append(nc.dram_tensor(f"wdown_{i}", [IHP, 512], F32R, kind="ExternalInput"))
    for i in range(ntab):
        ropeq_d.append(nc.dram_tensor(f"ropeq_{i}", [128, 1024], F32, kind="ExternalInput"))
        ropek_d.append(nc.dram_tensor(f"ropek_{i}", [128, 1024], F32, kind="ExternalInput"))
    out_d = nc.dram_tensor("out_tok", [NTOK, PD], F32, kind="ExternalOutput")

    with tile.TileContext(nc) as tc:
        with (
            tc.tile_pool(name="persist", bufs=1) as pp,
            tc.tile_pool(name="wqkp", bufs=1) as wqkp,
            tc.tile_pool(name="wvp", bufs=1) as wvp,
            tc.tile_pool(name="wop", bufs=1) as wop,
            tc.tile_pool(name="wgp", bufs=2) as wgp,
            tc.tile_pool(name="wdp", bufs=1) as wdp,
            tc.tile_pool(name="tabp", bufs=1) as tabp,
            tc.tile_pool(name="hp", bufs=2) as hp,
            tc.tile_pool(name="scr", bufs=2) as scr,
            tc.tile_pool(name="atp", bufs=1) as atp,
            tc.tile_pool(name="expp", bufs=2) as expp,
            tc.tile_pool(name="ps", bufs=2, space="PSUM") as ps,
            tc.tile_pool(name="psa", bufs=1, space="PSUM") as psa,
        ):
            x_all = pp.tile([128, DT, NTOK], F32)
            consts = pp.tile([128, 2], F32R)
            allones = pp.tile([128, 128], F32R)
            oneblocks = pp.tile([128, 192], F32R)
            nc.sync.dma_start(out=allones, in_=allones_d[:, :])
            nc.sync.dma_start(out=oneblocks, in_=oneblocks_d[:, :])
            nc.scalar.add_instruction(mybir.InstLoadActFuncSet(
                name=nc.get_next_instruction_name(), act_func_set_id=6, ins=[], outs=[]))
            pswap = pp.tile([128, 128], F32R)
            bones = pp.tile([128, 128], F32R)
            nc.sync.dma_start(out=consts, in_=consts_d[:, :])
            nc.sync.dma_start(out=pswap, in_=pswap_d[:, :])
            nc.sync.dma_start(out=bones, in_=bones_d[:, :])
            v_pad = pp.tile([128, 4, 4, 192], F32R)
            nc.vector.memset(bass.AP(tensor=v_pad.tensor, offset=v_pad.offset + 64,
                                     ap=[v_pad.ap[0], [192, 16], [1, 64]]).bitcast(F32), 0.0)
            biases = pp.tile([128, 2], F32)
            nc.sync.dma_start(out=biases, in_=biases_d[:, :])
            hsel = pp.tile([128, 2], F32R)
            nc.sync.dma_start(out=hsel, in_=hsel_d[:, :])
            eps_b = biases[:, 0:1]
            eps64_b = biases[:, 1:2]

            def load_tables(i):
                tabs = tabp.tile([128, 4, 512], F32, tag="tabs")  # cq sq ck sk
                nc.sync.dma_start(out=tabs[:, 0:2, :], in_=ropeq_d[i].rearrange("p (a m) -> p a m", a=2))
                nc.sync.dma_start(out=tabs[:, 2:4, :], in_=ropek_d[i].rearrange("p (a m) -> p a m", a=2))
                return tabs

            tabs0 = load_tables(0) if same_tables else None

            # h chunk [128, DT, 512] = x[:, :, cs] * rms_inv (stats pre-broadcast
            # via all-ones matmul: every output row = column sum)
            def make_inv(c):
                cs = slice(512 * c, 512 * (c + 1))
                ssqbc = ps.tile([128, 512], F32, tag="mm")
                for t in range(DT):
                    xsq = scr.tile([128, 512], F32R, tag="sq", bufs=1)
                    nc.gpsimd.tensor_mul(xsq, x_all[:, t, cs], x_all[:, t, cs])
                    nc.tensor.matmul(ssqbc, allones, xsq, start=(t == 0), stop=(t == DT - 1))
                # 1/sqrt(v) = exp(-0.5 * ln(v)); ln and exp share an ACT table set
                lnv = scr.tile([128, 512], F32, tag="lnv", bufs=1)
                nc.scalar.activation(lnv, ssqbc, AF.Ln, scale=1.0 / D, bias=eps_b)
                invbc = scr.tile([128, 512], F32, tag="invbc", bufs=4)
                nc.scalar.activation(invbc, lnv, AF.Exp, scale=-0.5)
                return invbc

            def make_h(c, invbc):
                cs = slice(512 * c, 512 * (c + 1))
                h = hp.tile([128, DT, 512], F32R, tag="h")
                for t in range(DT):
                    nc.gpsimd.tensor_mul(h[:, t, :], x_all[:, t, cs], invbc)
                return h

            # ---- patch embed
            wpe_sb = pp.tile([128, 2, 512], F32R)
            nc.sync.dma_start(out=wpe_sb, in_=wpe_d.rearrange("(t p) m -> p t m", p=128))
            enorm_sb = pp.tile([128, DT], F32)
            nc.sync.dma_start(out=enorm_sb, in_=enorm_d[:, :])
            for c in range(4):
                cs = slice(512 * c, 512 * (c + 1))
                x0c = scr.tile([128, 2, 512], F32R, tag="x0c", bufs=1, space="SBUF")
                nc.sync.dma_start(out=x0c, in_=x0_d.rearrange("(t p) m -> p t m", p=128)[:, :, cs])
                xe_ps = []
                for dout in range(DT):
                    pst = psa.tile([128, 512], F32, tag=f"acc{dout}", name=f"xe{dout}")
                    for din in range(2):
                        nc.tensor.matmul(pst, wpe_sb[:, din, 128 * dout:128 * (dout + 1)],
                                         x0c[:, din, :], start=(din == 0), stop=(din == 1))
                    xe_ps.append(pst)
                ssqbc = ps.tile([128, 512], F32, tag="mm")
                for t in range(DT):
                    xsq = scr.tile([128, 512], F32R, tag="sq", bufs=1)
                    nc.scalar.activation(xsq, xe_ps[t], AF.Square)
                    nc.tensor.matmul(ssqbc, allones, xsq, start=(t == 0), stop=(t == DT - 1))
                lnv = scr.tile([128, 512], F32, tag="lnv", bufs=1)
                nc.scalar.activation(lnv, ssqbc, AF.Ln, scale=1.0 / D, bias=eps_b)
                invbc = scr.tile([128, 512], F32, tag="invbc", bufs=4)
                nc.scalar.activation(invbc, lnv, AF.Exp, scale=-0.5)
                for t in range(DT):
                    tmp = scr.tile([128, 512], F32, tag="lnv", bufs=1)
                    nc.vector.tensor_mul(tmp, xe_ps[t], invbc)
                    nc.vector.tensor_scalar_mul(x_all[:, t, cs], tmp, enorm_sb[:, t:t + 1])

            # ---- transformer layers
            for li in range(NL):
                tabs = tabs0 if same_tables else load_tables(li)
                wqk = wqkp.tile([128, DT, 1024], F32R, tag="wqk")
                nc.sync.dma_start(out=wqk, in_=wqk_d[li].rearrange("(t p) m -> p t m", p=128))
                wv = wvp.tile([128, DT, 512], F32R, tag="wv")
                nc.sync.dma_start(out=wv, in_=wv_d[li].rearrange("(t p) m -> p t m", p=128))
                wout = wop.tile([128, DT, 512], F32R, tag="wout")
                nc.sync.dma_start(out=wout, in_=wout_d[li].rearrange("(t p) m -> p t m", p=128))
                wdown = wdp.tile([128, 6, 512], F32R, tag="wdown")
                nc.sync.dma_start(out=wdown, in_=wdown_d[li].rearrange("(t p) m -> p t m", p=128))

                for b in range(BL):
                    h = make_h(b, make_inv(b))
                    # v token-major, padded layout: per (lt, f): [v_h0 | 0(64) | v_h1]
                    for lt in range(4):
                        vps = ps.tile([128, 512], F32, tag="mm")
                        for d in range(DT):
                            nc.tensor.matmul(vps, h[:, d, 128 * lt:128 * (lt + 1)],
                                             wv[:, d, :], start=(d == 0), stop=(d == DT - 1))
                        vdst = bass.AP(tensor=v_pad.tensor,
                                       offset=v_pad.offset + lt * 768,
                                       ap=[v_pad.ap[0], [192, 4], [128, 2], [1, 64]])
                        nc.scalar.copy(vdst, vps[:, :].rearrange("p (f a e) -> p f a e", f=4, a=2))

                    # invk accumulator [128(l), 4(ls), 8(h) x 2]
                    ikps = psa.tile([128, 4, NH], F32, tag="acc2")
                    invk_lh = atp.tile([128, 4, NH], F32, tag="invk_lh")

                    o_fm = atp.tile([128, DT, 512], F32R, tag="o_fm")
                    for f in range(DT):
                        # q, k raw (feature-major) for this feat tile
                        q_raw = scr.tile([128, 512], F32R, tag="q_raw", bufs=1)
                        k_raw = scr.tile([128, 512], F32R, tag="k_raw", bufs=1)
                        for which, dst in ((0, q_raw), (1, k_raw)):
                            qkps = ps.tile([128, 512], F32, tag="mm")
                            for d in range(DT):
                                nc.tensor.matmul(qkps, wqk[:, d, 512 * which + 128 * f:512 * which + 128 * (f + 1)],
                                                 h[:, d, :], start=(d == 0), stop=(d == DT - 1))
                            nc.scalar.copy(dst, qkps)

                        # invq pre-broadcast (includes 1/8 attn scale)
                        qsq = scr.tile([128, 512], F32R, tag="sq", bufs=1)
                        nc.gpsimd.tensor_mul(qsq, q_raw, q_raw)
                        sbc = ps.tile([128, 512], F32, tag="mm")
                        nc.tensor.matmul(sbc, bones, qsq, start=True, stop=True)
                        lnq = scr.tile([128, 512], F32, tag="lnv", bufs=1)
                        nc.scalar.activation(lnq, sbc, AF.Ln, bias=eps64_b)
                        invq_bc = scr.tile([128, 512], F32, tag="invq_bc")
                        nc.scalar.activation(invq_bc, lnq, AF.Exp, scale=-0.5)

                        # invk token-major (this tile's 2 heads)
                        ksq = scr.tile([128, 512], F32R, tag="sq2", bufs=1)
                        nc.gpsimd.tensor_mul(ksq, k_raw, k_raw)
                        for ls in range(4):
                            nc.tensor.matmul(
                                ikps[:, ls, 2 * f:2 * f + 2],
                                ksq[:, 128 * ls:128 * (ls + 1)],
                                hsel[:, 0:2],
                                start=True, stop=True)
                        lnk = scr.tile([128, 8], F32, tag="lnk", bufs=1)
                        nc.scalar.activation(lnk, ikps[:, :, 2 * f:2 * f + 2],
                                             AF.Ln, scale=1.0 / HD, bias=eps_b)
                        nc.scalar.activation(invk_lh[:, :, 2 * f:2 * f + 2], lnk, AF.Exp, scale=-0.5)

                        # rope
                        q_rope = scr.tile([128, 512], F32R, tag="q_rope", bufs=2)
                        k_rope = scr.tile([128, 512], F32R, tag="k_rope", bufs=2)
                        swq = ps.tile([128, 512], F32, tag="sps")
                        nc.tensor.matmul(swq, pswap, q_raw, start=True, stop=True)
                        qc = scr.tile([128, 512], F32, tag="qc", bufs=1)
                        nc.vector.tensor_mul(qc, q_raw, tabs[:, 0, :])
                        qs = scr.tile([128, 512], F32, tag="qs", bufs=1)
                        nc.vector.tensor_mul(qs, swq, tabs[:, 1, :])
                        nc.vector.tensor_add(qc, qc, qs)
                        nc.vector.tensor_mul(q_rope, qc, invq_bc)
                        swk = ps.tile([128, 512], F32, tag="sps")
                        nc.tensor.matmul(swk, pswap, k_raw, start=True, stop=True)
                        kc = scr.tile([128, 512], F32, tag="kc", bufs=1)
                        nc.gpsimd.tensor_mul(kc, k_raw, tabs[:, 2, :])
                        ks = scr.tile([128, 512], F32, tag="ks", bufs=1)
                        nc.vector.tensor_mul(ks, swk, tabs[:, 3, :])
                        nc.gpsimd.tensor_add(k_rope, kc, ks)

                        # attention for this tile's two heads; head hh lands on
                        # output rows [64*hh, 64*hh+64) via padded stationaries
                        ops = psa.tile([128, 512], F32, tag="acc0")
                        rps = psa.tile([128, 512], F32, tag="acc1")
                        for hh in range(2):
                            hidx = 2 * f + hh
                            hsl = slice(64 * hh, 64 * (hh + 1))
                            for t in range(4):
                                qcols = slice(128 * t, 512)
                                # fp32r needs moving dim >= 256 for full rate:
                                # widen reads to 256 with zeroed filler columns
                                mcols = slice(min(128 * t, 256), 512)
                                sps = ps.tile([128, 512], F32, tag="sps")
                                nc.tensor.matmul(sps[:, mcols],
                                                 k_rope[hsl, 128 * t:128 * (t + 1)],
                                                 q_rope[hsl, mcols],
                                                 start=True, stop=True)
                                expt = expp.tile([128, 512], F32R, tag="expt")
                                nc.scalar.activation(expt[:, qcols], sps[:, qcols], AF.Exp,
                                                     scale=invk_lh[:, t, hidx:hidx + 1])
                                nc.gpsimd.memset(
                                    expt[64:128, 128 * t:128 * t + 64].bitcast(F32), 0.0)
                                if t == 3:
                                    nc.gpsimd.memset(expt[:, 256:384].bitcast(F32), 0.0)
                                first = (hh == 0 and t == 0)
                                last = (hh == 1 and t == 3)
                                nc.tensor.matmul(rps[:, mcols],
                                                 oneblocks[:, 64 * hh:64 * hh + 128],
                                                 expt[:, mcols],
                                                 start=first, stop=last)
                                nc.tensor.matmul(ops[:, mcols],
                                                 v_pad[:, t, f, 64 * hh:64 * hh + 128],
                                                 expt[:, mcols],
                                                 start=first, stop=last)
                        rrec = scr.tile([128, 512], F32, tag="rrec", bufs=1)
                        nc.vector.reciprocal(rrec, rps)
                        nc.vector.tensor_mul(o_fm[:, f, :], ops, rrec)

                    # out proj + residual
                    bs = slice(512 * b, 512 * (b + 1))
                    for dout in range(DT):
                        xps = ps.tile([128, 512], F32, tag="mm")
                        for d in range(DT):
                            nc.tensor.matmul(xps, wout[:, d, 128 * dout:128 * (dout + 1)],
                                             o_fm[:, d, :], start=(d == 0), stop=(d == DT - 1))
                        nc.vector.tensor_add(x_all[:, dout, bs], xps, x_all[:, dout, bs])

                # ---- mlp (stats first so ln/exp cluster before silus)
                mlp_invs = [make_inv(c) for c in range(4)]
                for c in range(4):
                    cs = slice(512 * c, 512 * (c + 1))
                    h = make_h(c, mlp_invs[c])
                    dps = [psa.tile([128, 512], F32, tag=f"acc{t}", name=f"dps{t}") for t in range(DT)]
                    for f in range(6):
                        wgu = wgp.tile([128, DT, 512], F32R, tag="wgu")
                        nc.sync.dma_start(out=wgu, in_=wgu_d[li][f].rearrange("(t p) m -> p t m", p=128))

                        def gu_mm(j):
                            pps = ps.tile([128, 512], F32, tag="mm", name=f"gu{j}")
                            for d in range(DT):
                                nc.tensor.matmul(pps, wgu[:, d, 128 * j:128 * (j + 1)],
                                                 h[:, d, :], start=(d == 0), stop=(d == DT - 1))
                            return pps
                        # order: g1, silu, u1, mul, g2, silu, u2, mul -> <=2 live psums
                        p0 = gu_mm(0)
                        s1 = scr.tile([128, 512], F32, tag="s1", bufs=1)
                        nc.scalar.activation(s1, p0, AF.Silu)
                        p2 = gu_mm(2)
                        t1 = scr.tile([128, 512], F32, tag="t1", bufs=1)
                        nc.vector.tensor_mul(t1, s1, p2)
                        p1 = gu_mm(1)
                        s2 = scr.tile([128, 512], F32, tag="s2", bufs=1)
                        nc.scalar.activation(s2, p1, AF.Silu)
                        p3 = gu_mm(3)
                        t2 = scr.tile([128, 512], F32, tag="t2", bufs=1)
                        nc.vector.tensor_mul(t2, s2, p3)
                        ug = scr.tile([128, 512], F32R, tag="ug", bufs=1)
                        nc.gpsimd.tensor_add(ug, t1, t2)
                        for dout in range(DT):
                            nc.tensor.matmul(dps[dout], wdown[:, f, 128 * dout:128 * (dout + 1)],
                                             ug, start=(f == 0), stop=(f == 5))
                    for dout in range(DT):
                        nc.vector.tensor_add(x_all[:, dout, cs], dps[dout], x_all[:, dout, cs])

            # ---- final norm + head
            whead_sb = scr.tile([128, DT, 256], F32R, tag="whead", bufs=1)
            nc.sync.dma_start(out=whead_sb, in_=whead_d.rearrange("(t p) m -> p t m", p=128))
            head_invs = [make_inv(c) for c in range(4)]
            for c in range(4):
                h = make_h(c, head_invs[c])
                for lt in range(4):
                    hps = ps.tile([128, 256], F32, tag="mm")
                    for d in range(DT):
                        nc.tensor.matmul(hps, h[:, d, 128 * lt:128 * (lt + 1)],
                                         whead_sb[:, d, :], start=(d == 0), stop=(d == DT - 1))
                    osb = scr.tile([128, PD], F32, tag="osb")
                    nc.scalar.copy(osb, hps[:, 0:PD])
                    nc.sync.dma_start(out=out_d[512 * c + 128 * lt:512 * c + 128 * (lt + 1), :], in_=osb)

    nc.finalize()
    return nc


# ---------------------------------------------------------------- entry

_CACHE = {}


def kernel(frames, params):
    frames = np.asarray(frames, dtype=np.float32)
    prep = host_prep(params)
    same_tables = prep["same_tables"]

    if "nc" not in _CACHE:
        _CACHE["nc"] = build_nc(same_tables=same_tables)
    nc = _CACHE["nc"]

    x0 = _patchify(frames)
    shared = {k: v for k, v in prep.items() if isinstance(v, np.ndarray)}
    in_maps = []
    for core in range(NCORES):
        m = dict(shared)
        xb = x0[core * BL:(core + 1) * BL].reshape(NTOK, PD)
        x0f = np.zeros((256, NTOK), np.float32)
        x0f[:PD] = xb.T
        m["x0"] = x0f
        in_maps.append(m)

    res = run_bass_kernel_spmd(nc, in_maps, core_ids=list(range(NCORES)))
    outs = []
    for core in range(NCORES):
        tok = res.results[core]["out_tok"].reshape(BL, L, PD)
        outs.append(_unpatchify(tok))
    return np.concatenate(outs, axis=0)


# revision 29
# speedup vs baseline: 1.0066x; 1.0066x over previous
"""Trainium2 Bass kernel for nn_ARVideoPatchTransformer_80436147519663.

Distribution: data-parallel over batch (B=32 -> 4 samples/core x 8 cores),
no collectives. Each core runs the full transformer on its shard.

On-chip dataflow (per core):
- Residual x is feature-major [D(4x128 partition-tiles), 2048 tokens], fp32.
- All matmuls run in float32r (full PE rate at N>=256, ~14-bit mantissa),
  accumulating fp32 in PSUM.
- QKV emits q,k feature-major (scores need hd on partitions) and v
  token-major (AV matmul wants l_k on partitions). No PE transposes anywhere.
- RoPE pair-swap = PE permutation matmul; rope mul/add on DVE (q) and
  GPSIMD (k) to balance engines.
- Softmax in scoresT orientation [l_k, l_q]: block-causal mask realized by
  restricting matmul column ranges + one 64x64 corner memset per tile;
  per-head q-rms (with 1/sqrt(HD) folded) is produced pre-broadcast by a
  block-ones stationary matmul; k-rms lands token-major via stationary-k^2
  matmuls and is folded into the exp() activation scale. Softmax denominators
  come from a 64-col ones matmul (pre-broadcast), one reciprocal per 2 heads.
"""
import math
import numpy as np

import concourse.bass as bass
from concourse import bacc
import concourse.mybir as mybir
import concourse.tile as tile
from concourse.bass_utils import run_bass_kernel_spmd

F32 = mybir.dt.float32
F32R = mybir.dt.float32r
AF = mybir.ActivationFunctionType
ALU = mybir.AluOpType

B, T, C, RES, P = 32, 8, 3, 64, 8
NP = (RES // P) ** 2          # 64
L = T * NP                    # 512
PD = C * P * P                # 192
D, NH, NL = 512, 8, 8
HD = D // NH                  # 64
RD = HD // 2                  # 32
INNER = 1364
IH = INNER // 2               # 682
IHP = 768                     # padded half (6*128)
MAXT = T + 1
EPS = 1e-6
NCORES = 8
BL = B // NCORES              # 4
NTOK = BL * L                 # 2048
DT = D // 128                 # 4


# ---------------------------------------------------------------- host prep

def _sin_cos(rotary_dim, max_len, base=10000.0):
    inv = 1.0 / (base ** (np.arange(0, rotary_dim, 2, dtype=np.float32) / rotary_dim))
    ang = np.outer(np.arange(max_len, dtype=np.float32), inv)
    ang = np.repeat(ang, 2, axis=-1)
    return np.cos(ang).astype(np.float32), np.sin(ang).astype(np.float32)


def _build_rope_tables(q_scale, k_scale):
    """[128(2 heads' feats), 512(l)] C and S tables with scale folded."""
    t_cos, t_sin = _sin_cos(RD, MAXT)
    s_cos, s_sin = _sin_cos(RD, NP)
    l = np.arange(L)
    cos_full = np.concatenate([t_cos[l // NP], s_cos[l % NP]], axis=1)  # [512, 64]
    sin_full = np.concatenate([t_sin[l // NP], s_sin[l % NP]], axis=1)

    def fold(s):
        s = np.asarray(s, np.float32)
        Cm = cos_full * s[None, :]
        Sm = np.empty_like(sin_full)
        Sm[:, 0::2] = -sin_full[:, 0::2] * s[None, 1::2]
        Sm[:, 1::2] = sin_full[:, 1::2] * s[None, 0::2]
        return np.tile(Cm.T, (2, 1)).copy(), np.tile(Sm.T, (2, 1)).copy()  # [128, 512]

    return fold(q_scale), fold(k_scale)


def _patchify(frames):
    b, t, c, hh, ww = frames.shape
    h = hh // P
    x = frames.reshape(b * t, c, h, P, h, P)
    x = x.transpose(0, 2, 4, 1, 3, 5)
    return x.reshape(b, t * h * h, c * P * P)


def _unpatchify(tokens):
    b, l, _ = tokens.shape
    h = RES // P
    t = l // (h * h)
    x = tokens.reshape(b * t, h, h, C, P, P)
    x = x.transpose(0, 3, 1, 4, 2, 5)
    return x.reshape(b, t, C, h * P, h * P)


def host_prep(params):
    def np32(a):
        return np.ascontiguousarray(np.asarray(a, dtype=np.float32))

    out = {}
    wpe = np.zeros((256, 512), np.float32)
    wpe[:192] = np32(params["patch_embed"])
    out["wpe"] = wpe
    out["enorm"] = np32(params["embed_norm"]).reshape(DT, 128).T.copy()  # [128, 4]

    qs0 = np32(params["layers"][0]["q_scale"])
    ks0 = np32(params["layers"][0]["k_scale"])
    same_tables = all(
        np.array_equal(np32(lp["q_scale"]), qs0) and np.array_equal(np32(lp["k_scale"]), ks0)
        for lp in params["layers"])
    out["same_tables"] = same_tables

    for i, lp in enumerate(params["layers"]):
        wqkv = np32(lp["qkv"]) * np32(lp["norm1"])[:, None]
        out[f"wqk_{i}"] = np.ascontiguousarray(wqkv[:, :1024])
        out[f"wv_{i}"] = np.ascontiguousarray(wqkv[:, 1024:])
        out[f"wout_{i}"] = np32(lp["out"])
        gate = np32(lp["gate"]) * np32(lp["norm2"])[:, None]
        up = np32(lp["up"]) * np32(lp["norm2"])[:, None]
        wgu = np.zeros((6, 512, 512), np.float32)
        for f in range(6):
            lo, hi = f * 128, min(f * 128 + 128, IH)
            if hi > lo:
                for j, src in enumerate((gate[:, :IH], gate[:, IH:], up[:, :IH], up[:, IH:])):
                    wgu[f, :, j * 128:j * 128 + hi - lo] = src[:, lo:hi]
        out[f"wgu_{i}"] = wgu
        wdown = np.zeros((IHP, 512), np.float32)
        wdown[:IH] = np32(lp["down"])
        out[f"wdown_{i}"] = wdown
        if i == 0 or not same_tables:
            (cq, sq), (ck, sk) = _build_rope_tables(lp["q_scale"], lp["k_scale"])
            out[f"ropeq_{i}"] = np.concatenate([cq, sq], axis=1)   # [128, 1024]
            out[f"ropek_{i}"] = np.concatenate([ck, sk], axis=1)

    whead = np.zeros((512, 256), np.float32)
    whead[:, :192] = np32(params["out_norm"])[:, None] * np32(params["head"])
    out["whead"] = whead

    consts = np.zeros((128, 2), np.float32)
    consts[:, :] = 1.0
    out["consts"] = consts
    out["allones"] = np.ones((128, 128), np.float32)
    oneblocks = np.zeros((128, 192), np.float32)
    oneblocks[:, 0:64] = 1.0
    oneblocks[:, 128:192] = 1.0
    out["oneblocks"] = oneblocks
    hsel = np.zeros((128, 2), np.float32)
    hsel[:64, 0] = 1.0
    hsel[64:, 1] = 1.0
    out["hsel"] = hsel
    biases = np.zeros((128, 2), np.float32)
    biases[:, 0] = EPS
    biases[:, 1] = HD * EPS
    out["biases"] = biases
    psw = np.zeros((128, 128), np.float32)
    for i in range(64):
        psw[2 * i, 2 * i + 1] = 1.0
        psw[2 * i + 1, 2 * i] = 1.0
    out["pswap"] = psw
    bones = np.zeros((128, 128), np.float32)
    bones[:64, :64] = 1.0
    bones[64:, 64:] = 1.0
    out["bones"] = bones
    return out


# ---------------------------------------------------------------- bass build

def _patch_act_tables():
    """Restrict the act-table chooser to {natural_log_exp_and_others, silu_and_others}
    so the greedy first-match picker stops thrashing between per-function sets.
    Indices are preserved (walrus maps set-id -> table by position)."""
    import concourse.hw_specs as hw_specs
    if getattr(hw_specs, "_act_tables_patched", False):
        return
    orig = hw_specs.get_activation_tables

    def patched(module_arch):
        tabs = orig(module_arch)
        keep = {"natural_log_exp_and_others", "silu_and_others"}
        return {k: (v if k in keep else set()) for k, v in tabs.items()}

    hw_specs.get_activation_tables = patched
    bacc.get_activation_tables = patched
    hw_specs._act_tables_patched = True


def build_nc(same_tables=True):
    _patch_act_tables()
    nc = bacc.Bacc()

    x0_d = nc.dram_tensor("x0", [256, NTOK], F32R, kind="ExternalInput")
    wpe_d = nc.dram_tensor("wpe", [256, 512], F32R, kind="ExternalInput")
    enorm_d = nc.dram_tensor("enorm", [128, DT], F32, kind="ExternalInput")
    whead_d = nc.dram_tensor("whead", [512, 256], F32R, kind="ExternalInput")
    consts_d = nc.dram_tensor("consts", [128, 2], F32R, kind="ExternalInput")
    allones_d = nc.dram_tensor("allones", [128, 128], F32R, kind="ExternalInput")
    oneblocks_d = nc.dram_tensor("oneblocks", [128, 192], F32R, kind="ExternalInput")
    pswap_d = nc.dram_tensor("pswap", [128, 128], F32R, kind="ExternalInput")
    bones_d = nc.dram_tensor("bones", [128, 128], F32R, kind="ExternalInput")
    biases_d = nc.dram_tensor("biases", [128, 2], F32, kind="ExternalInput")
    hsel_d = nc.dram_tensor("hsel", [128, 2], F32R, kind="ExternalInput")
    wqk_d, wv_d, wout_d, wgu_d, wdown_d, ropeq_d, ropek_d = [], [], [], [], [], [], []
    ntab = 1 if same_tables else NL
    for i in range(NL):
        wqk_d.append(nc.dram_tensor(f"wqk_{i}", [512, 1024], F32R, kind="ExternalInput"))
        wv_d.append(nc.dram_tensor(f"wv_{i}", [512, 512], F32R, kind="ExternalInput"))
        wout_d.append(nc.dram_tensor(f"wout_{i}", [512, 512], F32R, kind="ExternalInput"))
        wgu_d.append(nc.dram_tensor(f"wgu_{i}", [6, 512, 512], F32R, kind="ExternalInput"))
        wdown_d.append(nc.dram_tensor(f"wdown_{i}", [IHP, 512], F32R, kind="ExternalInput"))
    for i in range(ntab):
        ropeq_d.append(nc.dram_tensor(f"ropeq_{i}", [128, 1024], F32, kind="ExternalInput"))
        ropek_d.append(nc.dram_tensor(f"ropek_{i}", [128, 1024], F32, kind="ExternalInput"))
    out_d = nc.dram_tensor("out_tok", [NTOK, PD], F32, kind="ExternalOutput")

    with tile.TileContext(nc) as tc:
        with (
            tc.tile_pool(name="persist", bufs=1) as pp,
            tc.tile_pool(name="wqkp", bufs=1) as wqkp,
            tc.tile_pool(name="wvp", bufs=1) as wvp,
            tc.tile_pool(name="wop", bufs=1) as wop,
            tc.tile_pool(name="wgp", bufs=2) as wgp,
            tc.tile_pool(name="wdp", bufs=1) as wdp,
            tc.tile_pool(name="tabp", bufs=1) as tabp,
            tc.tile_pool(name="hp", bufs=2) as hp,
            tc.tile_pool(name="scr", bufs=2) as scr,
            tc.tile_pool(name="atp", bufs=1) as atp,
            tc.tile_pool(name="expp", bufs=2) as expp,
            tc.tile_pool(name="ps", bufs=2, space="PSUM") as ps,
            tc.tile_pool(name="psa", bufs=1, space="PSUM") as psa,
        ):
            x_all = pp.tile([128, DT, NTOK], F32)
            consts = pp.tile([128, 2], F32R)
            allones = pp.tile([128, 128], F32R)
            oneblocks = pp.tile([128, 192], F32R)
            nc.sync.dma_start(out=allones, in_=allones_d[:, :])
            nc.sync.dma_start(out=oneblocks, in_=oneblocks_d[:, :])
            nc.scalar.add_instruction(mybir.InstLoadActFuncSet(
                name=nc.get_next_instruction_name(), act_func_set_id=6, ins=[], outs=[]))
            pswap = pp.tile([128, 128], F32R)
            bones = pp.tile([128, 128], F32R)
            nc.sync.dma_start(out=consts, in_=consts_d[:, :])
            nc.sync.dma_start(out=pswap, in_=pswap_d[:, :])
            nc.sync.dma_start(out=bones, in_=bones_d[:, :])
            v_pad = pp.tile([128, 4, 4, 192], F32R)
            nc.vector.memset(bass.AP(tensor=v_pad.tensor, offset=v_pad.offset + 64,
                                     ap=[v_pad.ap[0], [192, 16], [1, 64]]).bitcast(F32), 0.0)
            biases = pp.tile([128, 2], F32)
            nc.sync.dma_start(out=biases, in_=biases_d[:, :])
            hsel = pp.tile([128, 2], F32R)
            nc.sync.dma_start(out=hsel, in_=hsel_d[:, :])
            eps_b = biases[:, 0:1]
            eps64_b = biases[:, 1:2]

            def load_tables(i):
                tabs = tabp.tile([128, 4, 512], F32, tag="tabs")  # cq sq ck sk
                nc.sync.dma_start(out=tabs[:, 0:2, :], in_=ropeq_d[i].rearrange("p (a m) -> p a m", a=2))
                nc.sync.dma_start(out=tabs[:, 2:4, :], in_=ropek_d[i].rearrange("p (a m) -> p a m", a=2))
                return tabs

            tabs0 = load_tables(0) if same_tables else None

            # h chunk [128, DT, 512] = x[:, :, cs] * rms_inv (stats pre-broadcast
            # via all-ones matmul: every output row = column sum)
            def make_inv(c):
                cs = slice(512 * c, 512 * (c + 1))
                ssqbc = ps.tile([128, 512], F32, tag="mm")
                for t in range(DT):
                    xsq = scr.tile([128, 512], F32R, tag="sq", bufs=1)
                    nc.gpsimd.tensor_mul(xsq, x_all[:, t, cs], x_all[:, t, cs])
                    nc.tensor.matmul(ssqbc, allones, xsq, start=(t == 0), stop=(t == DT - 1))
                # 1/sqrt(v) = exp(-0.5 * ln(v)); ln and exp share an ACT table set
                lnv = scr.tile([128, 512], F32, tag="lnv", bufs=1)
                nc.scalar.activation(lnv, ssqbc, AF.Ln, scale=1.0 / D, bias=eps_b)
                invbc = scr.tile([128, 512], F32, tag="invbc", bufs=4)
                nc.scalar.activation(invbc, lnv, AF.Exp, scale=-0.5)
                return invbc

            def make_h(c, invbc):
                cs = slice(512 * c, 512 * (c + 1))
                h = hp.tile([128, DT, 512], F32R, tag="h")
                for t in range(DT):
                    nc.gpsimd.tensor_mul(h[:, t, :], x_all[:, t, cs], invbc)
                return h

            # ---- patch embed
            wpe_sb = pp.tile([128, 2, 512], F32R)
            nc.sync.dma_start(out=wpe_sb, in_=wpe_d.rearrange("(t p) m -> p t m", p=128))
            enorm_sb = pp.tile([128, DT], F32)
            nc.sync.dma_start(out=enorm_sb, in_=enorm_d[:, :])
            for c in range(4):
                cs = slice(512 * c, 512 * (c + 1))
                x0c = scr.tile([128, 2, 512], F32R, tag="x0c", bufs=1, space="SBUF")
                nc.sync.dma_start(out=x0c, in_=x0_d.rearrange("(t p) m -> p t m", p=128)[:, :, cs])
                xe_ps = []
                for dout in range(DT):
                    pst = psa.tile([128, 512], F32, tag=f"acc{dout}", name=f"xe{dout}")
                    for din in range(2):
                        nc.tensor.matmul(pst, wpe_sb[:, din, 128 * dout:128 * (dout + 1)],
                                         x0c[:, din, :], start=(din == 0), stop=(din == 1))
                    xe_ps.append(pst)
                ssqbc = ps.tile([128, 512], F32, tag="mm")
                for t in range(DT):
                    xsq = scr.tile([128, 512], F32R, tag="sq", bufs=1)
                    nc.scalar.activation(xsq, xe_ps[t], AF.Square)
                    nc.tensor.matmul(ssqbc, allones, xsq, start=(t == 0), stop=(t == DT - 1))
                lnv = scr.tile([128, 512], F32, tag="lnv", bufs=1)
                nc.scalar.activation(lnv, ssqbc, AF.Ln, scale=1.0 / D, bias=eps_b)
                invbc = scr.tile([128, 512], F32, tag="invbc", bufs=4)
                nc.scalar.activation(invbc, lnv, AF.Exp, scale=-0.5)
                for t in range(DT):
                    tmp = scr.tile([128, 512], F32, tag="lnv", bufs=1)
                    nc.vector.tensor_mul(tmp, xe_ps[t], invbc)
                    nc.vector.tensor_scalar_mul(x_all[:, t, cs], tmp, enorm_sb[:, t:t + 1])

            # ---- transformer layers
            for li in range(NL):
                tabs = tabs0 if same_tables else load_tables(li)
                wqk = wqkp.tile([128, DT, 1024], F32R, tag="wqk")
                nc.sync.dma_start(out=wqk, in_=wqk_d[li].rearrange("(t p) m -> p t m", p=128))
                wv = wvp.tile([128, DT, 512], F32R, tag="wv")
                nc.sync.dma_start(out=wv, in_=wv_d[li].rearrange("(t p) m -> p t m", p=128))
                wout = wop.tile([128, DT, 512], F32R, tag="wout")
                nc.sync.dma_start(out=wout, in_=wout_d[li].rearrange("(t p) m -> p t m", p=128))
                wdown = wdp.tile([128, 6, 512], F32R, tag="wdown")
                nc.sync.dma_start(out=wdown, in_=wdown_d[li].rearrange("(t p) m -> p t m", p=128))

                for b in range(BL):
                    h = make_h(b, make_inv(b))
                    # v token-major, padded layout: per (lt, f): [v_h0 | 0(64) | v_h1]
                    for lt in range(4):
                        vps = ps.tile([128, 512], F32, tag="mm")
                        for d in range(DT):
                            nc.tensor.matmul(vps, h[:, d, 128 * lt:128 * (lt + 1)],
                                             wv[:, d, :], start=(d == 0), stop=(d == DT - 1))
                        vdst = bass.AP(tensor=v_pad.tensor,
                                       offset=v_pad.offset + lt * 768,
                                       ap=[v_pad.ap[0], [192, 4], [128, 2], [1, 64]])
                        nc.scalar.copy(vdst, vps[:, :].rearrange("p (f a e) -> p f a e", f=4, a=2))

                    # invk accumulator [128(l), 4(ls), 8(h) x 2]
                    ikps = psa.tile([128, 4, NH], F32, tag="acc2")
                    invk_lh = atp.tile([128, 4, NH], F32, tag="invk_lh")

                    o_fm = atp.tile([128, DT, 512], F32R, tag="o_fm")
                    for f in range(DT):
                        # q, k raw (feature-major) for this feat tile
                        q_raw = scr.tile([128, 512], F32R, tag="q_raw", bufs=1)
                        k_raw = scr.tile([128, 512], F32R, tag="k_raw", bufs=1)
                        for which, dst in ((0, q_raw), (1, k_raw)):
                            qkps = ps.tile([128, 512], F32, tag="mm")
                            for d in range(DT):
                                nc.tensor.matmul(qkps, wqk[:, d, 512 * which + 128 * f:512 * which + 128 * (f + 1)],
                                                 h[:, d, :], start=(d == 0), stop=(d == DT - 1))
                            nc.scalar.copy(dst, qkps)

                        # invq pre-broadcast (includes 1/8 attn scale)
                        qsq = scr.tile([128, 512], F32R, tag="sq", bufs=1)
                        nc.gpsimd.tensor_mul(qsq, q_raw, q_raw)
                        sbc = ps.tile([128, 512], F32, tag="mm")
                        nc.tensor.matmul(sbc, bones, qsq, start=True, stop=True)
                        lnq = scr.tile([128, 512], F32, tag="lnv", bufs=1)
                        nc.scalar.activation(lnq, sbc, AF.Ln, bias=eps64_b)
                        invq_bc = scr.tile([128, 512], F32, tag="invq_bc")
                        nc.scalar.activation(invq_bc, lnq, AF.Exp, scale=-0.5)

                        # invk token-major (this tile's 2 heads)
                        ksq = scr.tile([128, 512], F32R, tag="sq2", bufs=1)
                        nc.gpsimd.tensor_mul(ksq, k_raw, k_raw)
                        for ls in range(4):
                            nc.tensor.matmul(
                                ikps[:, ls, 2 * f:2 * f + 2],
                                ksq[:, 128 * ls:128 * (ls + 1)],
                                hsel[:, 0:2],
                                start=True, stop=True)
                        lnk = scr.tile([128, 8], F32, tag="lnk", bufs=1)
                        nc.scalar.activation(lnk, ikps[:, :, 2 * f:2 * f + 2],
                                             AF.Ln, scale=1.0 / HD, bias=eps_b)
                        nc.scalar.activation(invk_lh[:, :, 2 * f:2 * f + 2], lnk, AF.Exp, scale=-0.5)

                        # rope
                        q_rope = scr.tile([128, 512], F32R, tag="q_rope", bufs=2)
                        k_rope = scr.tile([128, 512], F32R, tag="k_rope", bufs=2)
                        swq = ps.tile([128, 512], F32, tag="sps")
                        nc.tensor.matmul(swq, pswap, q_raw, start=True, stop=True)
                        qc = scr.tile([128, 512], F32, tag="qc", bufs=1)
                        nc.vector.tensor_mul(qc, q_raw, tabs[:, 0, :])
                        qs = scr.tile([128, 512], F32, tag="qs", bufs=1)
                        nc.vector.tensor_mul(qs, swq, tabs[:, 1, :])
                        nc.vector.tensor_add(qc, qc, qs)
                        nc.vector.tensor_mul(q_rope, qc, invq_bc)
                        swk = ps.tile([128, 512], F32, tag="sps")
                        nc.tensor.matmul(swk, pswap, k_raw, start=True, stop=True)
                        kc = scr.tile([128, 512], F32, tag="kc", bufs=1)
                        nc.gpsimd.tensor_mul(kc, k_raw, tabs[:, 2, :])
                        ks = scr.tile([128, 512], F32, tag="ks", bufs=1)
                        nc.vector.tensor_mul(ks, swk, tabs[:, 3, :])
                        nc.gpsimd.tensor_add(k_rope, kc, ks)

                        # attention for this tile's two heads; head hh lands on
                        # output rows [64*hh, 64*hh+64) via padded stationaries
                        ops = psa.tile([128, 512], F32, tag="acc0" if f % 2 == 0 else "acc3")
                        rps = psa.tile([128, 512], F32, tag="acc1")
                        for t in range(4):
                            for hh in range(2):
                                hidx = 2 * f + hh
                                hsl = slice(64 * hh, 64 * (hh + 1))
                                qcols = slice(128 * t, 512)
                                # fp32r needs moving dim >= 256 for full rate:
                                # widen reads to 256 with zeroed filler columns
                                mcols = slice(min(128 * t, 256), 512)
                                sps = ps.tile([128, 512], F32, tag="sps")
                                nc.tensor.matmul(sps[:, mcols],
                                                 k_rope[hsl, 128 * t:128 * (t + 1)],
                                                 q_rope[hsl, mcols],
                                                 start=True, stop=True)
                                expt = expp.tile([128, 512], F32R, tag="expt")
                                nc.scalar.activation(expt[:, qcols], sps[:, qcols], AF.Exp,
                                                     scale=invk_lh[:, t, hidx:hidx + 1])
                                nc.gpsimd.memset(
                                    expt[64:128, 128 * t:128 * t + 64].bitcast(F32), 0.0)
                                if t == 3:
                                    nc.gpsimd.memset(expt[:, 256:384].bitcast(F32), 0.0)
                                first = (hh == 0 and t == 0)
                                last = (hh == 1 and t == 3)
                                nc.tensor.matmul(rps[:, mcols],
                                                 oneblocks[:, 64 * hh:64 * hh + 128],
                                                 expt[:, mcols],
                                                 start=first, stop=last)
                                nc.tensor.matmul(ops[:, mcols],
                                                 v_pad[:, t, f, 64 * hh:64 * hh + 128],
                                                 expt[:, mcols],
                                                 start=first, stop=last)
                        rrec = scr.tile([128, 512], F32, tag="rrec", bufs=1)
                        nc.vector.reciprocal(rrec, rps)
                        nc.vector.tensor_mul(o_fm[:, f, :], ops, rrec)

                    # out proj + residual
                    bs = slice(512 * b, 512 * (b + 1))
                    for dout in range(DT):
                        xps = ps.tile([128, 512], F32, tag="mm")
                        for d in range(DT):
                            nc.tensor.matmul(xps, wout[:, d, 128 * dout:128 * (dout + 1)],
                                             o_fm[:, d, :], start=(d == 0), stop=(d == DT - 1))
                        nc.vector.tensor_add(x_all[:, dout, bs], xps, x_all[:, dout, bs])

                # ---- mlp (stats first so ln/exp cluster before silus)
                mlp_invs = [make_inv(c) for c in range(4)]
                for c in range(4):
                    cs = slice(512 * c, 512 * (c + 1))
                    h = make_h(c, mlp_invs[c])
                    dps = [psa.tile([128, 512], F32, tag=f"acc{t}", name=f"dps{t}") for t in range(DT)]
                    for f in range(6):
                        wgu = wgp.tile([128, DT, 512], F32R, tag="wgu")
                        nc.sync.dma_start(out=wgu, in_=wgu_d[li][f].rearrange("(t p) m -> p t m", p=128))

                        def gu_mm(j):
                            pps = ps.tile([128, 512], F32, tag="mm", name=f"gu{j}")
                            for d in range(DT):
                                nc.tensor.matmul(pps, wgu[:, d, 128 * j:128 * (j + 1)],
                                                 h[:, d, :], start=(d == 0), stop=(d == DT - 1))
                            return pps
                        # order: g1, silu, u1, mul, g2, silu, u2, mul -> <=2 live psums
                        p0 = gu_mm(0)
                        s1 = scr.tile([128, 512], F32, tag="s1", bufs=1)
                        nc.scalar.activation(s1, p0, AF.Silu)
                        p2 = gu_mm(2)
                        t1 = scr.tile([128, 512], F32, tag="t1", bufs=1)
                        nc.vector.tensor_mul(t1, s1, p2)
                        p1 = gu_mm(1)
                        s2 = scr.tile([128, 512], F32, tag="s2", bufs=1)
                        nc.scalar.activation(s2, p1, AF.Silu)
                        p3 = gu_mm(3)
                        t2 = scr.tile([128, 512], F32, tag="t2", bufs=1)
                        nc.vector.tensor_mul(t2, s2, p3)
                        ug = scr.tile([128, 512], F32R, tag="ug", bufs=1)
                        nc.gpsimd.tensor_add(ug, t1, t2)
                        for dout in range(DT):
                            nc.tensor.matmul(dps[dout], wdown[:, f, 128 * dout:128 * (dout + 1)],
                                             ug, start=(f == 0), stop=(f == 5))
                    for dout in range(DT):
                        nc.vector.tensor_add(x_all[:, dout, cs], dps[dout], x_all[:, dout, cs])

            # ---- final norm + head
            whead_sb = scr.tile([128, DT, 256], F32R, tag="whead", bufs=1)
            nc.sync.dma_start(out=whead_sb, in_=whead_d.rearrange("(t p) m -> p t m", p=128))
            head_invs = [make_inv(c) for c in range(4)]
            for c in range(4):
                h = make_h(c, head_invs[c])
                for lt in range(4):
                    hps = ps.tile([128, 256], F32, tag="mm")
                    for d in range(DT):
                        nc.tensor.matmul(hps, h[:, d, 128 * lt:128 * (lt + 1)],
                                         whead_sb[:, d, :], start=(d == 0), stop=(d == DT - 1))
                    osb = scr.tile([128, PD], F32, tag="osb")
                    nc.scalar.copy(osb, hps[:, 0:PD])
                    nc.sync.dma_start(out=out_d[512 * c + 128 * lt:512 * c + 128 * (lt + 1), :], in_=osb)

    nc.finalize()
    return nc


# ---------------------------------------------------------------- entry

_CACHE = {}


def kernel(frames, params):
    frames = np.asarray(frames, dtype=np.float32)
    prep = host_prep(params)
    same_tables = prep["same_tables"]

    if "nc" not in _CACHE:
        _CACHE["nc"] = build_nc(same_tables=same_tables)
    nc = _CACHE["nc"]

    x0 = _patchify(frames)
    shared = {k: v for k, v in prep.items() if isinstance(v, np.ndarray)}
    in_maps = []
    for core in range(NCORES):
        m = dict(shared)
        xb = x0[core * BL:(core + 1) * BL].reshape(NTOK, PD)
        x0f = np.zeros((256, NTOK), np.float32)
        x0f[:PD] = xb.T
        m["x0"] = x0f
        in_maps.append(m)

    res = run_bass_kernel_spmd(nc, in_maps, core_ids=list(range(NCORES)))
    outs = []
    for core in range(NCORES):
        tok = res.results[core]["out_tok"].reshape(BL, L, PD)
        outs.append(_unpatchify(tok))
    return np.concatenate(outs, axis=0)


# revision 31
# speedup vs baseline: 1.0248x; 1.0181x over previous
"""Trainium2 Bass kernel for nn_ARVideoPatchTransformer_80436147519663.

Distribution: data-parallel over batch (B=32 -> 4 samples/core x 8 cores),
no collectives. Each core runs the full transformer on its shard.

On-chip dataflow (per core):
- Residual x is feature-major [D(4x128 partition-tiles), 2048 tokens], fp32.
- All matmuls run in float32r (full PE rate at N>=256, ~14-bit mantissa),
  accumulating fp32 in PSUM.
- QKV emits q,k feature-major (scores need hd on partitions) and v
  token-major (AV matmul wants l_k on partitions). No PE transposes anywhere.
- RoPE pair-swap = PE permutation matmul; rope mul/add on DVE (q) and
  GPSIMD (k) to balance engines.
- Softmax in scoresT orientation [l_k, l_q]: block-causal mask realized by
  restricting matmul column ranges + one 64x64 corner memset per tile;
  per-head q-rms (with 1/sqrt(HD) folded) is produced pre-broadcast by a
  block-ones stationary matmul; k-rms lands token-major via stationary-k^2
  matmuls and is folded into the exp() activation scale. Softmax denominators
  come from a 64-col ones matmul (pre-broadcast), one reciprocal per 2 heads.
"""
import math
import numpy as np

import concourse.bass as bass
from concourse import bacc
import concourse.mybir as mybir
import concourse.tile as tile
from concourse.bass_utils import run_bass_kernel_spmd

F32 = mybir.dt.float32
F32R = mybir.dt.float32r
AF = mybir.ActivationFunctionType
ALU = mybir.AluOpType

B, T, C, RES, P = 32, 8, 3, 64, 8
NP = (RES // P) ** 2          # 64
L = T * NP                    # 512
PD = C * P * P                # 192
D, NH, NL = 512, 8, 8
HD = D // NH                  # 64
RD = HD // 2                  # 32
INNER = 1364
IH = INNER // 2               # 682
IHP = 768                     # padded half (6*128)
MAXT = T + 1
EPS = 1e-6
NCORES = 8
BL = B // NCORES              # 4
NTOK = BL * L                 # 2048
DT = D // 128                 # 4


# ---------------------------------------------------------------- host prep

def _sin_cos(rotary_dim, max_len, base=10000.0):
    inv = 1.0 / (base ** (np.arange(0, rotary_dim, 2, dtype=np.float32) / rotary_dim))
    ang = np.outer(np.arange(max_len, dtype=np.float32), inv)
    ang = np.repeat(ang, 2, axis=-1)
    return np.cos(ang).astype(np.float32), np.sin(ang).astype(np.float32)


def _build_rope_tables(q_scale, k_scale):
    """[128(2 heads' feats), 512(l)] C and S tables with scale folded."""
    t_cos, t_sin = _sin_cos(RD, MAXT)
    s_cos, s_sin = _sin_cos(RD, NP)
    l = np.arange(L)
    cos_full = np.concatenate([t_cos[l // NP], s_cos[l % NP]], axis=1)  # [512, 64]
    sin_full = np.concatenate([t_sin[l // NP], s_sin[l % NP]], axis=1)

    def fold(s):
        s = np.asarray(s, np.float32)
        Cm = cos_full * s[None, :]
        Sm = np.empty_like(sin_full)
        Sm[:, 0::2] = -sin_full[:, 0::2] * s[None, 1::2]
        Sm[:, 1::2] = sin_full[:, 1::2] * s[None, 0::2]
        return np.tile(Cm.T, (2, 1)).copy(), np.tile(Sm.T, (2, 1)).copy()  # [128, 512]

    return fold(q_scale), fold(k_scale)


def _patchify(frames):
    b, t, c, hh, ww = frames.shape
    h = hh // P
    x = frames.reshape(b * t, c, h, P, h, P)
    x = x.transpose(0, 2, 4, 1, 3, 5)
    return x.reshape(b, t * h * h, c * P * P)


def _unpatchify(tokens):
    b, l, _ = tokens.shape
    h = RES // P
    t = l // (h * h)
    x = tokens.reshape(b * t, h, h, C, P, P)
    x = x.transpose(0, 3, 1, 4, 2, 5)
    return x.reshape(b, t, C, h * P, h * P)


def host_prep(params):
    def np32(a):
        return np.ascontiguousarray(np.asarray(a, dtype=np.float32))

    out = {}
    wpe = np.zeros((256, 512), np.float32)
    wpe[:192] = np32(params["patch_embed"])
    out["wpe"] = wpe
    out["enorm"] = np32(params["embed_norm"]).reshape(DT, 128).T.copy()  # [128, 4]

    qs0 = np32(params["layers"][0]["q_scale"])
    ks0 = np32(params["layers"][0]["k_scale"])
    same_tables = all(
        np.array_equal(np32(lp["q_scale"]), qs0) and np.array_equal(np32(lp["k_scale"]), ks0)
        for lp in params["layers"])
    out["same_tables"] = same_tables

    for i, lp in enumerate(params["layers"]):
        wqkv = np32(lp["qkv"]) * np32(lp["norm1"])[:, None]
        out[f"wqk_{i}"] = np.ascontiguousarray(wqkv[:, :1024])
        out[f"wv_{i}"] = np.ascontiguousarray(wqkv[:, 1024:])
        out[f"wout_{i}"] = np32(lp["out"])
        gate = np32(lp["gate"]) * np32(lp["norm2"])[:, None]
        up = np32(lp["up"]) * np32(lp["norm2"])[:, None]
        wgu = np.zeros((6, 512, 512), np.float32)
        for f in range(6):
            lo, hi = f * 128, min(f * 128 + 128, IH)
            if hi > lo:
                for j, src in enumerate((gate[:, :IH], gate[:, IH:], up[:, :IH], up[:, IH:])):
                    wgu[f, :, j * 128:j * 128 + hi - lo] = src[:, lo:hi]
        out[f"wgu_{i}"] = wgu
        wdown = np.zeros((IHP, 512), np.float32)
        wdown[:IH] = np32(lp["down"])
        out[f"wdown_{i}"] = wdown
        if i == 0 or not same_tables:
            (cq, sq), (ck, sk) = _build_rope_tables(lp["q_scale"], lp["k_scale"])
            out[f"ropeq_{i}"] = np.concatenate([cq, sq], axis=1)   # [128, 1024]
            out[f"ropek_{i}"] = np.concatenate([ck, sk], axis=1)

    whead = np.zeros((512, 256), np.float32)
    whead[:, :192] = np32(params["out_norm"])[:, None] * np32(params["head"])
    out["whead"] = whead

    consts = np.zeros((128, 2), np.float32)
    consts[:, :] = 1.0
    out["consts"] = consts
    out["allones"] = np.ones((128, 128), np.float32)
    oneblocks = np.zeros((128, 192), np.float32)
    oneblocks[:, 0:64] = 1.0
    oneblocks[:, 128:192] = 1.0
    out["oneblocks"] = oneblocks
    hsel = np.zeros((128, 2), np.float32)
    hsel[:64, 0] = 1.0
    hsel[64:, 1] = 1.0
    out["hsel"] = hsel
    biases = np.zeros((128, 2), np.float32)
    biases[:, 0] = EPS
    biases[:, 1] = HD * EPS
    out["biases"] = biases
    psw = np.zeros((128, 128), np.float32)
    for i in range(64):
        psw[2 * i, 2 * i + 1] = 1.0
        psw[2 * i + 1, 2 * i] = 1.0
    out["pswap"] = psw
    bones = np.zeros((128, 128), np.float32)
    bones[:64, :64] = 1.0
    bones[64:, 64:] = 1.0
    out["bones"] = bones
    return out


# ---------------------------------------------------------------- bass build

def _patch_act_tables():
    """Restrict the act-table chooser to {natural_log_exp_and_others, silu_and_others}
    so the greedy first-match picker stops thrashing between per-function sets.
    Indices are preserved (walrus maps set-id -> table by position)."""
    import concourse.hw_specs as hw_specs
    if getattr(hw_specs, "_act_tables_patched", False):
        return
    orig = hw_specs.get_activation_tables

    def patched(module_arch):
        tabs = orig(module_arch)
        keep = {"natural_log_exp_and_others", "silu_and_others"}
        return {k: (v if k in keep else set()) for k, v in tabs.items()}

    hw_specs.get_activation_tables = patched
    bacc.get_activation_tables = patched
    hw_specs._act_tables_patched = True


def build_nc(same_tables=True):
    _patch_act_tables()
    nc = bacc.Bacc()

    x0_d = nc.dram_tensor("x0", [256, NTOK], F32R, kind="ExternalInput")
    wpe_d = nc.dram_tensor("wpe", [256, 512], F32R, kind="ExternalInput")
    enorm_d = nc.dram_tensor("enorm", [128, DT], F32, kind="ExternalInput")
    whead_d = nc.dram_tensor("whead", [512, 256], F32R, kind="ExternalInput")
    consts_d = nc.dram_tensor("consts", [128, 2], F32R, kind="ExternalInput")
    allones_d = nc.dram_tensor("allones", [128, 128], F32R, kind="ExternalInput")
    oneblocks_d = nc.dram_tensor("oneblocks", [128, 192], F32R, kind="ExternalInput")
    pswap_d = nc.dram_tensor("pswap", [128, 128], F32R, kind="ExternalInput")
    bones_d = nc.dram_tensor("bones", [128, 128], F32R, kind="ExternalInput")
    biases_d = nc.dram_tensor("biases", [128, 2], F32, kind="ExternalInput")
    hsel_d = nc.dram_tensor("hsel", [128, 2], F32R, kind="ExternalInput")
    wqk_d, wv_d, wout_d, wgu_d, wdown_d, ropeq_d, ropek_d = [], [], [], [], [], [], []
    ntab = 1 if same_tables else NL
    for i in range(NL):
        wqk_d.append(nc.dram_tensor(f"wqk_{i}", [512, 1024], F32R, kind="ExternalInput"))
        wv_d.append(nc.dram_tensor(f"wv_{i}", [512, 512], F32R, kind="ExternalInput"))
        wout_d.append(nc.dram_tensor(f"wout_{i}", [512, 512], F32R, kind="ExternalInput"))
        wgu_d.append(nc.dram_tensor(f"wgu_{i}", [6, 512, 512], F32R, kind="ExternalInput"))
        wdown_d.append(nc.dram_tensor(f"wdown_{i}", [IHP, 512], F32R, kind="ExternalInput"))
    for i in range(ntab):
        ropeq_d.append(nc.dram_tensor(f"ropeq_{i}", [128, 1024], F32, kind="ExternalInput"))
        ropek_d.append(nc.dram_tensor(f"ropek_{i}", [128, 1024], F32, kind="ExternalInput"))
    out_d = nc.dram_tensor("out_tok", [NTOK, PD], F32, kind="ExternalOutput")

    with tile.TileContext(nc) as tc:
        with (
            tc.tile_pool(name="persist", bufs=1) as pp,
            tc.tile_pool(name="wqkp", bufs=1) as wqkp,
            tc.tile_pool(name="wvp", bufs=1) as wvp,
            tc.tile_pool(name="wop", bufs=1) as wop,
            tc.tile_pool(name="wgp", bufs=2) as wgp,
            tc.tile_pool(name="wdp", bufs=1) as wdp,
            tc.tile_pool(name="tabp", bufs=1) as tabp,
            tc.tile_pool(name="hp", bufs=2) as hp,
            tc.tile_pool(name="scr", bufs=2) as scr,
            tc.tile_pool(name="atp", bufs=1) as atp,
            tc.tile_pool(name="expp", bufs=3) as expp,
            tc.tile_pool(name="ps", bufs=2, space="PSUM") as ps,
            tc.tile_pool(name="psa", bufs=1, space="PSUM") as psa,
        ):
            x_all = pp.tile([128, DT, NTOK], F32)
            consts = pp.tile([128, 2], F32R)
            allones = pp.tile([128, 128], F32R)
            oneblocks = pp.tile([128, 192], F32R)
            nc.sync.dma_start(out=allones, in_=allones_d[:, :])
            nc.sync.dma_start(out=oneblocks, in_=oneblocks_d[:, :])
            nc.scalar.add_instruction(mybir.InstLoadActFuncSet(
                name=nc.get_next_instruction_name(), act_func_set_id=6, ins=[], outs=[]))
            pswap = pp.tile([128, 128], F32R)
            bones = pp.tile([128, 128], F32R)
            nc.sync.dma_start(out=consts, in_=consts_d[:, :])
            nc.sync.dma_start(out=pswap, in_=pswap_d[:, :])
            nc.sync.dma_start(out=bones, in_=bones_d[:, :])
            v_pad = pp.tile([128, 4, 4, 192], F32R)
            nc.vector.memset(bass.AP(tensor=v_pad.tensor, offset=v_pad.offset + 64,
                                     ap=[v_pad.ap[0], [192, 16], [1, 64]]).bitcast(F32), 0.0)
            biases = pp.tile([128, 2], F32)
            nc.sync.dma_start(out=biases, in_=biases_d[:, :])
            hsel = pp.tile([128, 2], F32R)
            nc.sync.dma_start(out=hsel, in_=hsel_d[:, :])
            eps_b = biases[:, 0:1]
            eps64_b = biases[:, 1:2]

            def load_tables(i):
                tabs = tabp.tile([128, 4, 512], F32, tag="tabs")  # cq sq ck sk
                nc.sync.dma_start(out=tabs[:, 0:2, :], in_=ropeq_d[i].rearrange("p (a m) -> p a m", a=2))
                nc.sync.dma_start(out=tabs[:, 2:4, :], in_=ropek_d[i].rearrange("p (a m) -> p a m", a=2))
                return tabs

            tabs0 = load_tables(0) if same_tables else None

            # h chunk [128, DT, 512] = x[:, :, cs] * rms_inv (stats pre-broadcast
            # via all-ones matmul: every output row = column sum)
            def make_inv(c):
                cs = slice(512 * c, 512 * (c + 1))
                ssqbc = ps.tile([128, 512], F32, tag="mm")
                for t in range(DT):
                    xsq = scr.tile([128, 512], F32R, tag="sq", bufs=1)
                    nc.gpsimd.tensor_mul(xsq, x_all[:, t, cs], x_all[:, t, cs])
                    nc.tensor.matmul(ssqbc, allones, xsq, start=(t == 0), stop=(t == DT - 1))
                # 1/sqrt(v) = exp(-0.5 * ln(v)); ln and exp share an ACT table set
                lnv = scr.tile([128, 512], F32, tag="lnv", bufs=1)
                nc.scalar.activation(lnv, ssqbc, AF.Ln, scale=1.0 / D, bias=eps_b)
                invbc = scr.tile([128, 512], F32, tag="invbc", bufs=4)
                nc.scalar.activation(invbc, lnv, AF.Exp, scale=-0.5)
                return invbc

            def make_h(c, invbc):
                cs = slice(512 * c, 512 * (c + 1))
                h = hp.tile([128, DT, 512], F32R, tag="h")
                for t in range(DT):
                    nc.gpsimd.tensor_mul(h[:, t, :], x_all[:, t, cs], invbc)
                return h

            # ---- patch embed
            wpe_sb = pp.tile([128, 2, 512], F32R)
            nc.sync.dma_start(out=wpe_sb, in_=wpe_d.rearrange("(t p) m -> p t m", p=128))
            enorm_sb = pp.tile([128, DT], F32)
            nc.sync.dma_start(out=enorm_sb, in_=enorm_d[:, :])
            for c in range(4):
                cs = slice(512 * c, 512 * (c + 1))
                x0c = scr.tile([128, 2, 512], F32R, tag="whead", bufs=1)
                nc.sync.dma_start(out=x0c, in_=x0_d.rearrange("(t p) m -> p t m", p=128)[:, :, cs])
                xe_ps = []
                for dout in range(DT):
                    pst = psa.tile([128, 512], F32, tag=f"acc{dout}", name=f"xe{dout}")
                    for din in range(2):
                        nc.tensor.matmul(pst, wpe_sb[:, din, 128 * dout:128 * (dout + 1)],
                                         x0c[:, din, :], start=(din == 0), stop=(din == 1))
                    xe_ps.append(pst)
                ssqbc = ps.tile([128, 512], F32, tag="mm")
                for t in range(DT):
                    xsq = scr.tile([128, 512], F32R, tag="sq", bufs=1)
                    nc.scalar.activation(xsq, xe_ps[t], AF.Square)
                    nc.tensor.matmul(ssqbc, allones, xsq, start=(t == 0), stop=(t == DT - 1))
                lnv = scr.tile([128, 512], F32, tag="lnv", bufs=1)
                nc.scalar.activation(lnv, ssqbc, AF.Ln, scale=1.0 / D, bias=eps_b)
                invbc = scr.tile([128, 512], F32, tag="invbc", bufs=4)
                nc.scalar.activation(invbc, lnv, AF.Exp, scale=-0.5)
                for t in range(DT):
                    tmp = scr.tile([128, 512], F32, tag="lnv", bufs=1)
                    nc.vector.tensor_mul(tmp, xe_ps[t], invbc)
                    nc.vector.tensor_scalar_mul(x_all[:, t, cs], tmp, enorm_sb[:, t:t + 1])

            # ---- transformer layers
            for li in range(NL):
                tabs = tabs0 if same_tables else load_tables(li)
                wqk = wqkp.tile([128, DT, 1024], F32R, tag="wqk")
                nc.sync.dma_start(out=wqk, in_=wqk_d[li].rearrange("(t p) m -> p t m", p=128))
                wv = wvp.tile([128, DT, 512], F32R, tag="wv")
                nc.sync.dma_start(out=wv, in_=wv_d[li].rearrange("(t p) m -> p t m", p=128))
                wout = wop.tile([128, DT, 512], F32R, tag="wout")
                nc.sync.dma_start(out=wout, in_=wout_d[li].rearrange("(t p) m -> p t m", p=128))
                wdown = wdp.tile([128, 6, 512], F32R, tag="wdown")
                nc.sync.dma_start(out=wdown, in_=wdown_d[li].rearrange("(t p) m -> p t m", p=128))

                for b in range(BL):
                    h = make_h(b, make_inv(b))
                    # v token-major, padded layout: per (lt, f): [v_h0 | 0(64) | v_h1]
                    for lt in range(4):
                        vps = ps.tile([128, 512], F32, tag="mm")
                        for d in range(DT):
                            nc.tensor.matmul(vps, h[:, d, 128 * lt:128 * (lt + 1)],
                                             wv[:, d, :], start=(d == 0), stop=(d == DT - 1))
                        vdst = bass.AP(tensor=v_pad.tensor,
                                       offset=v_pad.offset + lt * 768,
                                       ap=[v_pad.ap[0], [192, 4], [128, 2], [1, 64]])
                        nc.scalar.copy(vdst, vps[:, :].rearrange("p (f a e) -> p f a e", f=4, a=2))

                    # invk accumulator [128(l), 4(ls), 8(h) x 2]
                    ikps = psa.tile([128, 4, NH], F32, tag="acc2")
                    invk_lh = atp.tile([128, 4, NH], F32, tag="invk_lh")

                    o_fm = atp.tile([128, DT, 512], F32R, tag="o_fm")
                    for f in range(DT):
                        # q, k raw (feature-major) for this feat tile
                        q_raw = scr.tile([128, 512], F32R, tag="q_raw", bufs=1)
                        k_raw = scr.tile([128, 512], F32R, tag="k_raw", bufs=1)
                        for which, dst in ((0, q_raw), (1, k_raw)):
                            qkps = ps.tile([128, 512], F32, tag="mm")
                            for d in range(DT):
                                nc.tensor.matmul(qkps, wqk[:, d, 512 * which + 128 * f:512 * which + 128 * (f + 1)],
                                                 h[:, d, :], start=(d == 0), stop=(d == DT - 1))
                            nc.scalar.copy(dst, qkps)

                        # invq pre-broadcast (includes 1/8 attn scale)
                        qsq = scr.tile([128, 512], F32R, tag="sq", bufs=1)
                        nc.gpsimd.tensor_mul(qsq, q_raw, q_raw)
                        sbc = ps.tile([128, 512], F32, tag="mm")
                        nc.tensor.matmul(sbc, bones, qsq, start=True, stop=True)
                        lnq = scr.tile([128, 512], F32, tag="lnv", bufs=1)
                        nc.scalar.activation(lnq, sbc, AF.Ln, bias=eps64_b)
                        invq_bc = scr.tile([128, 512], F32, tag="invq_bc")
                        nc.scalar.activation(invq_bc, lnq, AF.Exp, scale=-0.5)

                        # invk token-major (this tile's 2 heads)
                        ksq = scr.tile([128, 512], F32R, tag="sq2", bufs=1)
                        nc.gpsimd.tensor_mul(ksq, k_raw, k_raw)
                        for ls in range(4):
                            nc.tensor.matmul(
                                ikps[:, ls, 2 * f:2 * f + 2],
                                ksq[:, 128 * ls:128 * (ls + 1)],
                                hsel[:, 0:2],
                                start=True, stop=True)
                        lnk = scr.tile([128, 8], F32, tag="lnk", bufs=1)
                        nc.scalar.activation(lnk, ikps[:, :, 2 * f:2 * f + 2],
                                             AF.Ln, scale=1.0 / HD, bias=eps_b)
                        nc.scalar.activation(invk_lh[:, :, 2 * f:2 * f + 2], lnk, AF.Exp, scale=-0.5)

                        # rope
                        q_rope = scr.tile([128, 512], F32R, tag="q_rope", bufs=2)
                        k_rope = scr.tile([128, 512], F32R, tag="k_rope", bufs=2)
                        swq = ps.tile([128, 512], F32, tag="sps")
                        nc.tensor.matmul(swq, pswap, q_raw, start=True, stop=True)
                        qc = scr.tile([128, 512], F32, tag="qc", bufs=1)
                        nc.vector.tensor_mul(qc, q_raw, tabs[:, 0, :])
                        qs = scr.tile([128, 512], F32, tag="qs", bufs=1)
                        nc.vector.tensor_mul(qs, swq, tabs[:, 1, :])
                        nc.vector.tensor_add(qc, qc, qs)
                        nc.vector.tensor_mul(q_rope, qc, invq_bc)
                        swk = ps.tile([128, 512], F32, tag="sps")
                        nc.tensor.matmul(swk, pswap, k_raw, start=True, stop=True)
                        kc = scr.tile([128, 512], F32, tag="kc", bufs=1)
                        nc.gpsimd.tensor_mul(kc, k_raw, tabs[:, 2, :])
                        ks = scr.tile([128, 512], F32, tag="ks", bufs=1)
                        nc.vector.tensor_mul(ks, swk, tabs[:, 3, :])
                        nc.gpsimd.tensor_add(k_rope, kc, ks)

                        # attention for this tile's two heads; head hh lands on
                        # output rows [64*hh, 64*hh+64) via padded stationaries
                        ops = psa.tile([128, 512], F32, tag="acc0" if f % 2 == 0 else "acc3")
                        rps = psa.tile([128, 512], F32, tag="acc1")
                        for t in range(4):
                            for hh in range(2):
                                hidx = 2 * f + hh
                                hsl = slice(64 * hh, 64 * (hh + 1))
                                qcols = slice(128 * t, 512)
                                # fp32r needs moving dim >= 256 for full rate:
                                # widen reads to 256 with zeroed filler columns
                                mcols = slice(min(128 * t, 256), 512)
                                sps = ps.tile([128, 512], F32, tag="sps")
                                nc.tensor.matmul(sps[:, mcols],
                                                 k_rope[hsl, 128 * t:128 * (t + 1)],
                                                 q_rope[hsl, mcols],
                                                 start=True, stop=True)
                                expt = expp.tile([128, 512], F32R, tag="expt")
                                nc.scalar.activation(expt[:, qcols], sps[:, qcols], AF.Exp,
                                                     scale=invk_lh[:, t, hidx:hidx + 1])
                                nc.gpsimd.memset(
                                    expt[64:128, 128 * t:128 * t + 64].bitcast(F32), 0.0)
                                if t == 3:
                                    nc.gpsimd.memset(expt[:, 256:384].bitcast(F32), 0.0)
                                first = (hh == 0 and t == 0)
                                last = (hh == 1 and t == 3)
                                nc.tensor.matmul(rps[:, mcols],
                                                 oneblocks[:, 64 * hh:64 * hh + 128],
                                                 expt[:, mcols],
                                                 start=first, stop=last)
                                nc.tensor.matmul(ops[:, mcols],
                                                 v_pad[:, t, f, 64 * hh:64 * hh + 128],
                                                 expt[:, mcols],
                                                 start=first, stop=last)
                        rrec = scr.tile([128, 512], F32, tag="rrec", bufs=1)
                        nc.vector.reciprocal(rrec, rps)
                        nc.vector.tensor_mul(o_fm[:, f, :], ops, rrec)

                    # out proj + residual
                    bs = slice(512 * b, 512 * (b + 1))
                    for dout in range(DT):
                        xps = ps.tile([128, 512], F32, tag="mm")
                        for d in range(DT):
                            nc.tensor.matmul(xps, wout[:, d, 128 * dout:128 * (dout + 1)],
                                             o_fm[:, d, :], start=(d == 0), stop=(d == DT - 1))
                        nc.vector.tensor_add(x_all[:, dout, bs], xps, x_all[:, dout, bs])

                # ---- mlp (stats first so ln/exp cluster before silus)
                mlp_invs = [make_inv(c) for c in range(4)]
                for c in range(4):
                    cs = slice(512 * c, 512 * (c + 1))
                    h = make_h(c, mlp_invs[c])
                    dps = [psa.tile([128, 512], F32, tag=f"acc{t}", name=f"dps{t}") for t in range(DT)]
                    for f in range(6):
                        wgu = wgp.tile([128, DT, 512], F32R, tag="wgu")
                        nc.sync.dma_start(out=wgu, in_=wgu_d[li][f].rearrange("(t p) m -> p t m", p=128))

                        def gu_mm(j):
                            pps = ps.tile([128, 512], F32, tag="mm", name=f"gu{j}")
                            for d in range(DT):
                                nc.tensor.matmul(pps, wgu[:, d, 128 * j:128 * (j + 1)],
                                                 h[:, d, :], start=(d == 0), stop=(d == DT - 1))
                            return pps
                        # order: g1, silu, u1, mul, g2, silu, u2, mul -> <=2 live psums
                        p0 = gu_mm(0)
                        s1 = scr.tile([128, 512], F32, tag="s1", bufs=1)
                        nc.scalar.activation(s1, p0, AF.Silu)
                        p2 = gu_mm(2)
                        t1 = scr.tile([128, 512], F32, tag="t1", bufs=1)
                        nc.vector.tensor_mul(t1, s1, p2)
                        p1 = gu_mm(1)
                        s2 = scr.tile([128, 512], F32, tag="s2", bufs=1)
                        nc.scalar.activation(s2, p1, AF.Silu)
                        p3 = gu_mm(3)
                        t2 = scr.tile([128, 512], F32, tag="t2", bufs=1)
                        nc.vector.tensor_mul(t2, s2, p3)
                        ug = scr.tile([128, 512], F32R, tag="ug", bufs=1)
                        nc.gpsimd.tensor_add(ug, t1, t2)
                        for dout in range(DT):
                            nc.tensor.matmul(dps[dout], wdown[:, f, 128 * dout:128 * (dout + 1)],
                                             ug, start=(f == 0), stop=(f == 5))
                    for dout in range(DT):
                        nc.vector.tensor_add(x_all[:, dout, cs], dps[dout], x_all[:, dout, cs])

            # ---- final norm + head
            whead_sb = scr.tile([128, DT, 256], F32R, tag="whead", bufs=1)
            nc.sync.dma_start(out=whead_sb, in_=whead_d.rearrange("(t p) m -> p t m", p=128))
            head_invs = [make_inv(c) for c in range(4)]
            for c in range(4):
                h = make_h(c, head_invs[c])
                for lt in range(4):
                    hps = ps.tile([128, 256], F32, tag="mm")
                    for d in range(DT):
                        nc.tensor.matmul(hps, h[:, d, 128 * lt:128 * (lt + 1)],
                                         whead_sb[:, d, :], start=(d == 0), stop=(d == DT - 1))
                    osb = scr.tile([128, PD], F32, tag="osb")
                    nc.scalar.copy(osb, hps[:, 0:PD])
                    nc.sync.dma_start(out=out_d[512 * c + 128 * lt:512 * c + 128 * (lt + 1), :], in_=osb)

    nc.finalize()
    return nc


# ---------------------------------------------------------------- entry

_CACHE = {}


def kernel(frames, params):
    frames = np.asarray(frames, dtype=np.float32)
    prep = host_prep(params)
    same_tables = prep["same_tables"]

    if "nc" not in _CACHE:
        _CACHE["nc"] = build_nc(same_tables=same_tables)
    nc = _CACHE["nc"]

    x0 = _patchify(frames)
    shared = {k: v for k, v in prep.items() if isinstance(v, np.ndarray)}
    in_maps = []
    for core in range(NCORES):
        m = dict(shared)
        xb = x0[core * BL:(core + 1) * BL].reshape(NTOK, PD)
        x0f = np.zeros((256, NTOK), np.float32)
        x0f[:PD] = xb.T
        m["x0"] = x0f
        in_maps.append(m)

    res = run_bass_kernel_spmd(nc, in_maps, core_ids=list(range(NCORES)))
    outs = []
    for core in range(NCORES):
        tok = res.results[core]["out_tok"].reshape(BL, L, PD)
        outs.append(_unpatchify(tok))
    return np.concatenate(outs, axis=0)


# revision 35
# speedup vs baseline: 1.0624x; 1.0367x over previous
"""Trainium2 Bass kernel for nn_ARVideoPatchTransformer_80436147519663.

Distribution: data-parallel over batch (B=32 -> 4 samples/core x 8 cores),
no collectives. Each core runs the full transformer on its shard.

On-chip dataflow (per core):
- Residual x is feature-major [D(4x128 partition-tiles), 2048 tokens], fp32.
- All matmuls run in float32r (full PE rate at N>=256, ~14-bit mantissa),
  accumulating fp32 in PSUM.
- QKV emits q,k feature-major (scores need hd on partitions) and v
  token-major (AV matmul wants l_k on partitions). No PE transposes anywhere.
- RoPE pair-swap = PE permutation matmul; rope mul/add on DVE (q) and
  GPSIMD (k) to balance engines.
- Softmax in scoresT orientation [l_k, l_q]: block-causal mask realized by
  restricting matmul column ranges + one 64x64 corner memset per tile;
  per-head q-rms (with 1/sqrt(HD) folded) is produced pre-broadcast by a
  block-ones stationary matmul; k-rms lands token-major via stationary-k^2
  matmuls and is folded into the exp() activation scale. Softmax denominators
  come from a 64-col ones matmul (pre-broadcast), one reciprocal per 2 heads.
"""
import math
import numpy as np

import concourse.bass as bass
from concourse import bacc
import concourse.mybir as mybir
import concourse.tile as tile
from concourse.bass_utils import run_bass_kernel_spmd

F32 = mybir.dt.float32
F32R = mybir.dt.float32r
AF = mybir.ActivationFunctionType
ALU = mybir.AluOpType

B, T, C, RES, P = 32, 8, 3, 64, 8
NP = (RES // P) ** 2          # 64
L = T * NP                    # 512
PD = C * P * P                # 192
D, NH, NL = 512, 8, 8
HD = D // NH                  # 64
RD = HD // 2                  # 32
INNER = 1364
IH = INNER // 2               # 682
IHP = 768                     # padded half (6*128)
MAXT = T + 1
EPS = 1e-6
NCORES = 8
BL = B // NCORES              # 4
NTOK = BL * L                 # 2048
DT = D // 128                 # 4


# ---------------------------------------------------------------- host prep

def _sin_cos(rotary_dim, max_len, base=10000.0):
    inv = 1.0 / (base ** (np.arange(0, rotary_dim, 2, dtype=np.float32) / rotary_dim))
    ang = np.outer(np.arange(max_len, dtype=np.float32), inv)
    ang = np.repeat(ang, 2, axis=-1)
    return np.cos(ang).astype(np.float32), np.sin(ang).astype(np.float32)


def _build_rope_tables(q_scale, k_scale):
    """[128(2 heads' feats), 512(l)] C and S tables with scale folded."""
    t_cos, t_sin = _sin_cos(RD, MAXT)
    s_cos, s_sin = _sin_cos(RD, NP)
    l = np.arange(L)
    cos_full = np.concatenate([t_cos[l // NP], s_cos[l % NP]], axis=1)  # [512, 64]
    sin_full = np.concatenate([t_sin[l // NP], s_sin[l % NP]], axis=1)

    def fold(s):
        s = np.asarray(s, np.float32)
        Cm = cos_full * s[None, :]
        Sm = np.empty_like(sin_full)
        Sm[:, 0::2] = -sin_full[:, 0::2] * s[None, 1::2]
        Sm[:, 1::2] = sin_full[:, 1::2] * s[None, 0::2]
        return np.tile(Cm.T, (2, 1)).copy(), np.tile(Sm.T, (2, 1)).copy()  # [128, 512]

    return fold(q_scale), fold(k_scale)


def _patchify(frames):
    b, t, c, hh, ww = frames.shape
    h = hh // P
    x = frames.reshape(b * t, c, h, P, h, P)
    x = x.transpose(0, 2, 4, 1, 3, 5)
    return x.reshape(b, t * h * h, c * P * P)


def _unpatchify(tokens):
    b, l, _ = tokens.shape
    h = RES // P
    t = l // (h * h)
    x = tokens.reshape(b * t, h, h, C, P, P)
    x = x.transpose(0, 3, 1, 4, 2, 5)
    return x.reshape(b, t, C, h * P, h * P)


def host_prep(params):
    def np32(a):
        return np.ascontiguousarray(np.asarray(a, dtype=np.float32))

    out = {}
    wpe = np.zeros((256, 512), np.float32)
    wpe[:192] = np32(params["patch_embed"])
    out["wpe"] = wpe
    out["enorm"] = np32(params["embed_norm"]).reshape(DT, 128).T.copy()  # [128, 4]

    qs0 = np32(params["layers"][0]["q_scale"])
    ks0 = np32(params["layers"][0]["k_scale"])
    same_tables = all(
        np.array_equal(np32(lp["q_scale"]), qs0) and np.array_equal(np32(lp["k_scale"]), ks0)
        for lp in params["layers"])
    out["same_tables"] = same_tables

    for i, lp in enumerate(params["layers"]):
        wqkv = np32(lp["qkv"]) * np32(lp["norm1"])[:, None]
        out[f"wqk_{i}"] = np.ascontiguousarray(wqkv[:, :1024])
        out[f"wv_{i}"] = np.ascontiguousarray(wqkv[:, 1024:])
        out[f"wout_{i}"] = np32(lp["out"])
        gate = np32(lp["gate"]) * np32(lp["norm2"])[:, None]
        up = np32(lp["up"]) * np32(lp["norm2"])[:, None]
        wgu = np.zeros((6, 512, 512), np.float32)
        for f in range(6):
            lo, hi = f * 128, min(f * 128 + 128, IH)
            if hi > lo:
                for j, src in enumerate((gate[:, :IH], gate[:, IH:], up[:, :IH], up[:, IH:])):
                    wgu[f, :, j * 128:j * 128 + hi - lo] = src[:, lo:hi]
        out[f"wgu_{i}"] = wgu
        wdown = np.zeros((IHP, 512), np.float32)
        wdown[:IH] = np32(lp["down"])
        out[f"wdown_{i}"] = wdown
        if i == 0 or not same_tables:
            (cq, sq), (ck, sk) = _build_rope_tables(lp["q_scale"], lp["k_scale"])
            out[f"ropeq_{i}"] = np.concatenate([cq, sq], axis=1)   # [128, 1024]
            out[f"ropek_{i}"] = np.concatenate([ck, sk], axis=1)

    whead = np.zeros((512, 256), np.float32)
    whead[:, :192] = np32(params["out_norm"])[:, None] * np32(params["head"])
    out["whead"] = whead

    consts = np.zeros((128, 2), np.float32)
    consts[:, :] = 1.0
    out["consts"] = consts
    out["allones"] = np.ones((128, 128), np.float32)
    oneblocks = np.zeros((128, 192), np.float32)
    oneblocks[:, 0:64] = 1.0
    oneblocks[:, 128:192] = 1.0
    out["oneblocks"] = oneblocks
    hsel = np.zeros((128, 2), np.float32)
    hsel[:64, 0] = 1.0
    hsel[64:, 1] = 1.0
    out["hsel"] = hsel
    biases = np.zeros((128, 2), np.float32)
    biases[:, 0] = EPS
    biases[:, 1] = HD * EPS
    out["biases"] = biases
    psw = np.zeros((128, 128), np.float32)
    for i in range(64):
        psw[2 * i, 2 * i + 1] = 1.0
        psw[2 * i + 1, 2 * i] = 1.0
    out["pswap"] = psw
    bones = np.zeros((128, 128), np.float32)
    bones[:64, :64] = 1.0
    bones[64:, 64:] = 1.0
    out["bones"] = bones
    return out


# ---------------------------------------------------------------- bass build

def _patch_act_tables():
    """Restrict the act-table chooser to {natural_log_exp_and_others, silu_and_others}
    so the greedy first-match picker stops thrashing between per-function sets.
    Indices are preserved (walrus maps set-id -> table by position)."""
    import concourse.hw_specs as hw_specs
    if getattr(hw_specs, "_act_tables_patched", False):
        return
    orig = hw_specs.get_activation_tables

    def patched(module_arch):
        tabs = orig(module_arch)
        keep = {"natural_log_exp_and_others", "silu_and_others"}
        return {k: (v if k in keep else set()) for k, v in tabs.items()}

    hw_specs.get_activation_tables = patched
    bacc.get_activation_tables = patched
    hw_specs._act_tables_patched = True


def build_nc(same_tables=True):
    _patch_act_tables()
    nc = bacc.Bacc()

    x0_d = nc.dram_tensor("x0", [256, NTOK], F32R, kind="ExternalInput")
    wpe_d = nc.dram_tensor("wpe", [256, 512], F32R, kind="ExternalInput")
    enorm_d = nc.dram_tensor("enorm", [128, DT], F32, kind="ExternalInput")
    whead_d = nc.dram_tensor("whead", [512, 256], F32R, kind="ExternalInput")
    consts_d = nc.dram_tensor("consts", [128, 2], F32R, kind="ExternalInput")
    allones_d = nc.dram_tensor("allones", [128, 128], F32R, kind="ExternalInput")
    oneblocks_d = nc.dram_tensor("oneblocks", [128, 192], F32R, kind="ExternalInput")
    pswap_d = nc.dram_tensor("pswap", [128, 128], F32R, kind="ExternalInput")
    bones_d = nc.dram_tensor("bones", [128, 128], F32R, kind="ExternalInput")
    biases_d = nc.dram_tensor("biases", [128, 2], F32, kind="ExternalInput")
    hsel_d = nc.dram_tensor("hsel", [128, 2], F32R, kind="ExternalInput")
    wqk_d, wv_d, wout_d, wgu_d, wdown_d, ropeq_d, ropek_d = [], [], [], [], [], [], []
    ntab = 1 if same_tables else NL
    for i in range(NL):
        wqk_d.append(nc.dram_tensor(f"wqk_{i}", [512, 1024], F32R, kind="ExternalInput"))
        wv_d.append(nc.dram_tensor(f"wv_{i}", [512, 512], F32R, kind="ExternalInput"))
        wout_d.append(nc.dram_tensor(f"wout_{i}", [512, 512], F32R, kind="ExternalInput"))
        wgu_d.append(nc.dram_tensor(f"wgu_{i}", [6, 512, 512], F32R, kind="ExternalInput"))
        wdown_d.append(nc.dram_tensor(f"wdown_{i}", [IHP, 512], F32R, kind="ExternalInput"))
    for i in range(ntab):
        ropeq_d.append(nc.dram_tensor(f"ropeq_{i}", [128, 1024], F32, kind="ExternalInput"))
        ropek_d.append(nc.dram_tensor(f"ropek_{i}", [128, 1024], F32, kind="ExternalInput"))
    out_d = nc.dram_tensor("out_tok", [NTOK, PD], F32, kind="ExternalOutput")

    with tile.TileContext(nc) as tc:
        with (
            tc.tile_pool(name="persist", bufs=1) as pp,
            tc.tile_pool(name="wqkp", bufs=1) as wqkp,
            tc.tile_pool(name="wvp", bufs=1) as wvp,
            tc.tile_pool(name="wop", bufs=1) as wop,
            tc.tile_pool(name="wgp", bufs=2) as wgp,
            tc.tile_pool(name="wdp", bufs=1) as wdp,
            tc.tile_pool(name="tabp", bufs=1) as tabp,
            tc.tile_pool(name="hp", bufs=2) as hp,
            tc.tile_pool(name="scr", bufs=2) as scr,
            tc.tile_pool(name="atp", bufs=1) as atp,
            tc.tile_pool(name="expp", bufs=3) as expp,
            tc.tile_pool(name="ps", bufs=2, space="PSUM") as ps,
            tc.tile_pool(name="psa", bufs=1, space="PSUM") as psa,
        ):
            x_all = pp.tile([128, DT, NTOK], F32)
            consts = pp.tile([128, 2], F32R)
            allones = pp.tile([128, 128], F32R)
            oneblocks = pp.tile([128, 192], F32R)
            nc.sync.dma_start(out=allones, in_=allones_d[:, :])
            nc.sync.dma_start(out=oneblocks, in_=oneblocks_d[:, :])
            nc.scalar.add_instruction(mybir.InstLoadActFuncSet(
                name=nc.get_next_instruction_name(), act_func_set_id=6, ins=[], outs=[]))
            pswap = pp.tile([128, 128], F32R)
            bones = pp.tile([128, 128], F32R)
            nc.sync.dma_start(out=consts, in_=consts_d[:, :])
            nc.sync.dma_start(out=pswap, in_=pswap_d[:, :])
            nc.sync.dma_start(out=bones, in_=bones_d[:, :])
            v_pad = pp.tile([128, 4, 4, 192], F32R)
            nc.vector.memset(bass.AP(tensor=v_pad.tensor, offset=v_pad.offset + 64,
                                     ap=[v_pad.ap[0], [192, 16], [1, 64]]).bitcast(F32), 0.0)
            biases = pp.tile([128, 2], F32)
            nc.sync.dma_start(out=biases, in_=biases_d[:, :])
            hsel = pp.tile([128, 2], F32R)
            nc.sync.dma_start(out=hsel, in_=hsel_d[:, :])
            eps_b = biases[:, 0:1]
            eps64_b = biases[:, 1:2]

            def load_tables(i):
                tabs = tabp.tile([128, 4, 512], F32, tag="tabs")  # cq sq ck sk
                nc.sync.dma_start(out=tabs[:, 0:2, :], in_=ropeq_d[i].rearrange("p (a m) -> p a m", a=2))
                nc.sync.dma_start(out=tabs[:, 2:4, :], in_=ropek_d[i].rearrange("p (a m) -> p a m", a=2))
                return tabs

            tabs0 = load_tables(0) if same_tables else None

            # h chunk [128, DT, 512] = x[:, :, cs] * rms_inv (stats pre-broadcast
            # via all-ones matmul: every output row = column sum)
            def make_inv(c):
                cs = slice(512 * c, 512 * (c + 1))
                ssqbc = ps.tile([128, 512], F32, tag="mm")
                for t in range(DT):
                    xsq = scr.tile([128, 512], F32R, tag="sq", bufs=2)
                    nc.gpsimd.tensor_mul(xsq, x_all[:, t, cs], x_all[:, t, cs])
                    nc.tensor.matmul(ssqbc, allones, xsq, start=(t == 0), stop=(t == DT - 1))
                # 1/sqrt(v) = exp(-0.5 * ln(v)); ln and exp share an ACT table set
                lnv = scr.tile([128, 512], F32, tag="lnv", bufs=1)
                nc.scalar.activation(lnv, ssqbc, AF.Ln, scale=1.0 / D, bias=eps_b)
                invbc = scr.tile([128, 512], F32, tag="invbc", bufs=4)
                nc.scalar.activation(invbc, lnv, AF.Exp, scale=-0.5)
                return invbc

            def make_h(c, invbc):
                cs = slice(512 * c, 512 * (c + 1))
                h = hp.tile([128, DT, 512], F32R, tag="h")
                for t in range(DT):
                    nc.gpsimd.tensor_mul(h[:, t, :], x_all[:, t, cs], invbc)
                return h

            # ---- patch embed
            wpe_sb = pp.tile([128, 2, 512], F32R)
            nc.sync.dma_start(out=wpe_sb, in_=wpe_d.rearrange("(t p) m -> p t m", p=128))
            enorm_sb = pp.tile([128, DT], F32)
            nc.sync.dma_start(out=enorm_sb, in_=enorm_d[:, :])
            for c in range(4):
                cs = slice(512 * c, 512 * (c + 1))
                x0c = scr.tile([128, 2, 512], F32R, tag="whead", bufs=1)
                nc.sync.dma_start(out=x0c, in_=x0_d.rearrange("(t p) m -> p t m", p=128)[:, :, cs])
                xe_ps = []
                for dout in range(DT):
                    pst = psa.tile([128, 512], F32, tag=f"acc{dout}", name=f"xe{dout}")
                    for din in range(2):
                        nc.tensor.matmul(pst, wpe_sb[:, din, 128 * dout:128 * (dout + 1)],
                                         x0c[:, din, :], start=(din == 0), stop=(din == 1))
                    xe_ps.append(pst)
                ssqbc = ps.tile([128, 512], F32, tag="mm")
                for t in range(DT):
                    xsq = scr.tile([128, 512], F32R, tag="sq", bufs=2)
                    nc.scalar.activation(xsq, xe_ps[t], AF.Square)
                    nc.tensor.matmul(ssqbc, allones, xsq, start=(t == 0), stop=(t == DT - 1))
                lnv = scr.tile([128, 512], F32, tag="lnv", bufs=1)
                nc.scalar.activation(lnv, ssqbc, AF.Ln, scale=1.0 / D, bias=eps_b)
                invbc = scr.tile([128, 512], F32, tag="invbc", bufs=4)
                nc.scalar.activation(invbc, lnv, AF.Exp, scale=-0.5)
                for t in range(DT):
                    tmp = scr.tile([128, 512], F32, tag="lnv", bufs=1)
                    nc.vector.tensor_mul(tmp, xe_ps[t], invbc)
                    nc.vector.tensor_scalar_mul(x_all[:, t, cs], tmp, enorm_sb[:, t:t + 1])

            # ---- transformer layers
            for li in range(NL):
                tabs = tabs0 if same_tables else load_tables(li)
                wqk = wqkp.tile([128, DT, 1024], F32R, tag="wqk")
                nc.sync.dma_start(out=wqk, in_=wqk_d[li].rearrange("(t p) m -> p t m", p=128))
                wv = wvp.tile([128, DT, 512], F32R, tag="wv")
                nc.sync.dma_start(out=wv, in_=wv_d[li].rearrange("(t p) m -> p t m", p=128))
                wout = wop.tile([128, DT, 512], F32R, tag="wout")
                nc.sync.dma_start(out=wout, in_=wout_d[li].rearrange("(t p) m -> p t m", p=128))
                wdown = wdp.tile([128, 6, 512], F32R, tag="wdown")
                nc.sync.dma_start(out=wdown, in_=wdown_d[li].rearrange("(t p) m -> p t m", p=128))

                for b in range(BL):
                    h = make_h(b, make_inv(b))
                    # v token-major, padded layout: per (lt, f): [v_h0 | 0(64) | v_h1]
                    for lt in range(4):
                        vps = ps.tile([128, 512], F32, tag="mm")
                        for d in range(DT):
                            nc.tensor.matmul(vps, h[:, d, 128 * lt:128 * (lt + 1)],
                                             wv[:, d, :], start=(d == 0), stop=(d == DT - 1))
                        vdst = bass.AP(tensor=v_pad.tensor,
                                       offset=v_pad.offset + lt * 768,
                                       ap=[v_pad.ap[0], [192, 4], [128, 2], [1, 64]])
                        nc.scalar.copy(vdst, vps[:, :].rearrange("p (f a e) -> p f a e", f=4, a=2))

                    # invk accumulator [128(l), 4(ls), 8(h) x 2]
                    ikps = psa.tile([128, 4, NH], F32, tag="acc2")
                    invk_lh = atp.tile([128, 4, NH], F32, tag="invk_lh")

                    o_fm = atp.tile([128, DT, 512], F32R, tag="o_fm")
                    for f in range(DT):
                        # q, k raw (feature-major) for this feat tile
                        q_raw = scr.tile([128, 512], F32R, tag="q_raw", bufs=1)
                        k_raw = scr.tile([128, 512], F32R, tag="k_raw", bufs=1)
                        for which, dst in ((0, q_raw), (1, k_raw)):
                            qkps = ps.tile([128, 512], F32, tag="mm")
                            for d in range(DT):
                                nc.tensor.matmul(qkps, wqk[:, d, 512 * which + 128 * f:512 * which + 128 * (f + 1)],
                                                 h[:, d, :], start=(d == 0), stop=(d == DT - 1))
                            nc.scalar.copy(dst, qkps)

                        # invq pre-broadcast (includes 1/8 attn scale)
                        qsq = scr.tile([128, 512], F32R, tag="sq", bufs=2)
                        nc.gpsimd.tensor_mul(qsq, q_raw, q_raw)
                        sbc = ps.tile([128, 512], F32, tag="mm")
                        nc.tensor.matmul(sbc, bones, qsq, start=True, stop=True)
                        lnq = scr.tile([128, 512], F32, tag="lnv", bufs=1)
                        nc.scalar.activation(lnq, sbc, AF.Ln, bias=eps64_b)
                        invq_bc = scr.tile([128, 512], F32, tag="invq_bc")
                        nc.scalar.activation(invq_bc, lnq, AF.Exp, scale=-0.5)

                        # invk token-major (this tile's 2 heads)
                        ksq = scr.tile([128, 512], F32R, tag="sq2", bufs=1)
                        nc.gpsimd.tensor_mul(ksq, k_raw, k_raw)
                        for ls in range(4):
                            nc.tensor.matmul(
                                ikps[:, ls, 2 * f:2 * f + 2],
                                ksq[:, 128 * ls:128 * (ls + 1)],
                                hsel[:, 0:2],
                                start=True, stop=True)
                        lnk = scr.tile([128, 8], F32, tag="lnk", bufs=1)
                        nc.scalar.activation(lnk, ikps[:, :, 2 * f:2 * f + 2],
                                             AF.Ln, scale=1.0 / HD, bias=eps_b)
                        nc.scalar.activation(invk_lh[:, :, 2 * f:2 * f + 2], lnk, AF.Exp, scale=-0.5)

                        # rope
                        q_rope = scr.tile([128, 512], F32R, tag="q_rope", bufs=2)
                        k_rope = scr.tile([128, 512], F32R, tag="k_rope", bufs=2)
                        swq = ps.tile([128, 512], F32, tag="sps")
                        nc.tensor.matmul(swq, pswap, q_raw, start=True, stop=True)
                        qc = scr.tile([128, 512], F32, tag="qc", bufs=1)
                        nc.vector.tensor_mul(qc, q_raw, tabs[:, 0, :])
                        qs = scr.tile([128, 512], F32, tag="qs", bufs=1)
                        nc.vector.tensor_mul(qs, swq, tabs[:, 1, :])
                        nc.vector.tensor_add(qc, qc, qs)
                        nc.vector.tensor_mul(q_rope, qc, invq_bc)
                        swk = ps.tile([128, 512], F32, tag="sps")
                        nc.tensor.matmul(swk, pswap, k_raw, start=True, stop=True)
                        kc = scr.tile([128, 512], F32, tag="kc", bufs=1)
                        nc.gpsimd.tensor_mul(kc, k_raw, tabs[:, 2, :])
                        ks = scr.tile([128, 512], F32, tag="ks", bufs=1)
                        nc.vector.tensor_mul(ks, swk, tabs[:, 3, :])
                        nc.gpsimd.tensor_add(k_rope, kc, ks)

                        # attention for this tile's two heads; head hh lands on
                        # output rows [64*hh, 64*hh+64) via padded stationaries
                        ops = psa.tile([128, 512], F32, tag="acc0" if f % 2 == 0 else "acc3")
                        rps = psa.tile([128, 512], F32, tag="acc1")
                        for t in range(4):
                            for hh in range(2):
                                hidx = 2 * f + hh
                                hsl = slice(64 * hh, 64 * (hh + 1))
                                qcols = slice(128 * t, 512)
                                # fp32r needs moving dim >= 256 for full rate:
                                # widen reads to 256 with zeroed filler columns
                                mcols = slice(min(128 * t, 256), 512)
                                sps = ps.tile([128, 512], F32, tag="sps")
                                nc.tensor.matmul(sps[:, mcols],
                                                 k_rope[hsl, 128 * t:128 * (t + 1)],
                                                 q_rope[hsl, mcols],
                                                 start=True, stop=True)
                                expt = expp.tile([128, 512], F32R, tag="expt")
                                nc.scalar.activation(expt[:, qcols], sps[:, qcols], AF.Exp,
                                                     scale=invk_lh[:, t, hidx:hidx + 1])
                                nc.gpsimd.memset(
                                    expt[64:128, 128 * t:128 * t + 64].bitcast(F32), 0.0)
                                if t == 3:
                                    nc.gpsimd.memset(expt[:, 256:384].bitcast(F32), 0.0)
                                first = (hh == 0 and t == 0)
                                last = (hh == 1 and t == 3)
                                nc.tensor.matmul(rps[:, mcols],
                                                 oneblocks[:, 64 * hh:64 * hh + 128],
                                                 expt[:, mcols],
                                                 start=first, stop=last)
                                nc.tensor.matmul(ops[:, mcols],
                                                 v_pad[:, t, f, 64 * hh:64 * hh + 128],
                                                 expt[:, mcols],
                                                 start=first, stop=last)
                        rrec = scr.tile([128, 512], F32, tag="rrec", bufs=1)
                        nc.vector.reciprocal(rrec, rps)
                        nc.vector.tensor_mul(o_fm[:, f, :], ops, rrec)

                    # out proj + residual
                    bs = slice(512 * b, 512 * (b + 1))
                    for dout in range(DT):
                        xps = ps.tile([128, 512], F32, tag="mm")
                        for d in range(DT):
                            nc.tensor.matmul(xps, wout[:, d, 128 * dout:128 * (dout + 1)],
                                             o_fm[:, d, :], start=(d == 0), stop=(d == DT - 1))
                        nc.vector.tensor_add(x_all[:, dout, bs], xps, x_all[:, dout, bs])

                # ---- mlp (stats first so ln/exp cluster before silus)
                mlp_invs = [make_inv(c) for c in range(4)]
                for c in range(4):
                    cs = slice(512 * c, 512 * (c + 1))
                    h = make_h(c, mlp_invs[c])
                    dps = [psa.tile([128, 512], F32, tag=f"acc{t}", name=f"dps{t}") for t in range(DT)]
                    for f in range(6):
                        wgu = wgp.tile([128, DT, 512], F32R, tag="wgu")
                        nc.sync.dma_start(out=wgu, in_=wgu_d[li][f].rearrange("(t p) m -> p t m", p=128))

                        def gu_mm(j):
                            pps = ps.tile([128, 512], F32, tag="mm", name=f"gu{j}")
                            for d in range(DT):
                                nc.tensor.matmul(pps, wgu[:, d, 128 * j:128 * (j + 1)],
                                                 h[:, d, :], start=(d == 0), stop=(d == DT - 1))
                            return pps
                        # order: g1, silu, u1, mul, g2, silu, u2, mul -> <=2 live psums
                        p0 = gu_mm(0)
                        s1 = scr.tile([128, 512], F32, tag="s1", bufs=1)
                        nc.scalar.activation(s1, p0, AF.Silu)
                        p2 = gu_mm(2)
                        t1 = scr.tile([128, 512], F32, tag="t1", bufs=1)
                        nc.vector.tensor_mul(t1, s1, p2)
                        p1 = gu_mm(1)
                        s2 = scr.tile([128, 512], F32, tag="s2", bufs=1)
                        nc.scalar.activation(s2, p1, AF.Silu)
                        p3 = gu_mm(3)
                        t2 = scr.tile([128, 512], F32, tag="t2", bufs=1)
                        nc.vector.tensor_mul(t2, s2, p3)
                        ug = scr.tile([128, 512], F32R, tag="ug", bufs=1)
                        nc.gpsimd.tensor_add(ug, t1, t2)
                        for dout in range(DT):
                            nc.tensor.matmul(dps[dout], wdown[:, f, 128 * dout:128 * (dout + 1)],
                                             ug, start=(f == 0), stop=(f == 5))
                    for dout in range(DT):
                        nc.vector.tensor_add(x_all[:, dout, cs], dps[dout], x_all[:, dout, cs])

            # ---- final norm + head
            whead_sb = scr.tile([128, DT, 256], F32R, tag="whead", bufs=1)
            nc.sync.dma_start(out=whead_sb, in_=whead_d.rearrange("(t p) m -> p t m", p=128))
            head_invs = [make_inv(c) for c in range(4)]
            for c in range(4):
                h = make_h(c, head_invs[c])
                for lt in range(4):
                    hps = ps.tile([128, 256], F32, tag="mm")
                    for d in range(DT):
                        nc.tensor.matmul(hps, h[:, d, 128 * lt:128 * (lt + 1)],
                                         whead_sb[:, d, :], start=(d == 0), stop=(d == DT - 1))
                    osb = scr.tile([128, PD], F32, tag="osb")
                    nc.scalar.copy(osb, hps[:, 0:PD])
                    nc.sync.dma_start(out=out_d[512 * c + 128 * lt:512 * c + 128 * (lt + 1), :], in_=osb)

    nc.finalize()
    return nc


# ---------------------------------------------------------------- entry

_CACHE = {}


def kernel(frames, params):
    frames = np.asarray(frames, dtype=np.float32)
    prep = host_prep(params)
    same_tables = prep["same_tables"]

    if "nc" not in _CACHE:
        _CACHE["nc"] = build_nc(same_tables=same_tables)
    nc = _CACHE["nc"]

    x0 = _patchify(frames)
    shared = {k: v for k, v in prep.items() if isinstance(v, np.ndarray)}
    in_maps = []
    for core in range(NCORES):
        m = dict(shared)
        xb = x0[core * BL:(core + 1) * BL].reshape(NTOK, PD)
        x0f = np.zeros((256, NTOK), np.float32)
        x0f[:PD] = xb.T
        m["x0"] = x0f
        in_maps.append(m)

    res = run_bass_kernel_spmd(nc, in_maps, core_ids=list(range(NCORES)))
    outs = []
    for core in range(NCORES):
        tok = res.results[core]["out_tok"].reshape(BL, L, PD)
        outs.append(_unpatchify(tok))
    return np.concatenate(outs, axis=0)


# revision 37
# speedup vs baseline: 1.1307x; 1.0643x over previous
"""Trainium2 Bass kernel for nn_ARVideoPatchTransformer_80436147519663.

Distribution: data-parallel over batch (B=32 -> 4 samples/core x 8 cores),
no collectives. Each core runs the full transformer on its shard.

On-chip dataflow (per core):
- Residual x is feature-major [D(4x128 partition-tiles), 2048 tokens], fp32.
- All matmuls run in float32r (full PE rate at N>=256, ~14-bit mantissa),
  accumulating fp32 in PSUM.
- QKV emits q,k feature-major (scores need hd on partitions) and v
  token-major (AV matmul wants l_k on partitions). No PE transposes anywhere.
- RoPE pair-swap = PE permutation matmul; rope mul/add on DVE (q) and
  GPSIMD (k) to balance engines.
- Softmax in scoresT orientation [l_k, l_q]: block-causal mask realized by
  restricting matmul column ranges + one 64x64 corner memset per tile;
  per-head q-rms (with 1/sqrt(HD) folded) is produced pre-broadcast by a
  block-ones stationary matmul; k-rms lands token-major via stationary-k^2
  matmuls and is folded into the exp() activation scale. Softmax denominators
  come from a 64-col ones matmul (pre-broadcast), one reciprocal per 2 heads.
"""
import math
import numpy as np

import concourse.bass as bass
from concourse import bacc
import concourse.mybir as mybir
import concourse.tile as tile
from concourse.bass_utils import run_bass_kernel_spmd

F32 = mybir.dt.float32
F32R = mybir.dt.float32r
AF = mybir.ActivationFunctionType
ALU = mybir.AluOpType

B, T, C, RES, P = 32, 8, 3, 64, 8
NP = (RES // P) ** 2          # 64
L = T * NP                    # 512
PD = C * P * P                # 192
D, NH, NL = 512, 8, 8
HD = D // NH                  # 64
RD = HD // 2                  # 32
INNER = 1364
IH = INNER // 2               # 682
IHP = 768                     # padded half (6*128)
MAXT = T + 1
EPS = 1e-6
NCORES = 8
BL = B // NCORES              # 4
NTOK = BL * L                 # 2048
DT = D // 128                 # 4


# ---------------------------------------------------------------- host prep

def _sin_cos(rotary_dim, max_len, base=10000.0):
    inv = 1.0 / (base ** (np.arange(0, rotary_dim, 2, dtype=np.float32) / rotary_dim))
    ang = np.outer(np.arange(max_len, dtype=np.float32), inv)
    ang = np.repeat(ang, 2, axis=-1)
    return np.cos(ang).astype(np.float32), np.sin(ang).astype(np.float32)


def _build_rope_tables(q_scale, k_scale):
    """[128(2 heads' feats), 512(l)] C and S tables with scale folded."""
    t_cos, t_sin = _sin_cos(RD, MAXT)
    s_cos, s_sin = _sin_cos(RD, NP)
    l = np.arange(L)
    cos_full = np.concatenate([t_cos[l // NP], s_cos[l % NP]], axis=1)  # [512, 64]
    sin_full = np.concatenate([t_sin[l // NP], s_sin[l % NP]], axis=1)

    def fold(s):
        s = np.asarray(s, np.float32)
        Cm = cos_full * s[None, :]
        Sm = np.empty_like(sin_full)
        Sm[:, 0::2] = -sin_full[:, 0::2] * s[None, 1::2]
        Sm[:, 1::2] = sin_full[:, 1::2] * s[None, 0::2]
        return np.tile(Cm.T, (2, 1)).copy(), np.tile(Sm.T, (2, 1)).copy()  # [128, 512]

    return fold(q_scale), fold(k_scale)


def _patchify(frames):
    b, t, c, hh, ww = frames.shape
    h = hh // P
    x = frames.reshape(b * t, c, h, P, h, P)
    x = x.transpose(0, 2, 4, 1, 3, 5)
    return x.reshape(b, t * h * h, c * P * P)


def _unpatchify(tokens):
    b, l, _ = tokens.shape
    h = RES // P
    t = l // (h * h)
    x = tokens.reshape(b * t, h, h, C, P, P)
    x = x.transpose(0, 3, 1, 4, 2, 5)
    return x.reshape(b, t, C, h * P, h * P)


def host_prep(params):
    def np32(a):
        return np.ascontiguousarray(np.asarray(a, dtype=np.float32))

    out = {}
    wpe = np.zeros((256, 512), np.float32)
    wpe[:192] = np32(params["patch_embed"])
    out["wpe"] = wpe
    out["enorm"] = np32(params["embed_norm"]).reshape(DT, 128).T.copy()  # [128, 4]

    qs0 = np32(params["layers"][0]["q_scale"])
    ks0 = np32(params["layers"][0]["k_scale"])
    same_tables = all(
        np.array_equal(np32(lp["q_scale"]), qs0) and np.array_equal(np32(lp["k_scale"]), ks0)
        for lp in params["layers"])
    out["same_tables"] = same_tables

    for i, lp in enumerate(params["layers"]):
        wqkv = np32(lp["qkv"]) * np32(lp["norm1"])[:, None]
        out[f"wqk_{i}"] = np.ascontiguousarray(wqkv[:, :1024])
        out[f"wv_{i}"] = np.ascontiguousarray(wqkv[:, 1024:])
        out[f"wout_{i}"] = np32(lp["out"])
        gate = np32(lp["gate"]) * np32(lp["norm2"])[:, None]
        up = np32(lp["up"]) * np32(lp["norm2"])[:, None]
        wgu = np.zeros((6, 512, 512), np.float32)
        for f in range(6):
            lo, hi = f * 128, min(f * 128 + 128, IH)
            if hi > lo:
                for j, src in enumerate((gate[:, :IH], gate[:, IH:], up[:, :IH], up[:, IH:])):
                    wgu[f, :, j * 128:j * 128 + hi - lo] = src[:, lo:hi]
        out[f"wgu_{i}"] = wgu
        wdown = np.zeros((IHP, 512), np.float32)
        wdown[:IH] = np32(lp["down"])
        out[f"wdown_{i}"] = wdown
        if i == 0 or not same_tables:
            (cq, sq), (ck, sk) = _build_rope_tables(lp["q_scale"], lp["k_scale"])
            out[f"ropeq_{i}"] = np.concatenate([cq, sq], axis=1)   # [128, 1024]
            out[f"ropek_{i}"] = np.concatenate([ck, sk], axis=1)

    whead = np.zeros((512, 256), np.float32)
    whead[:, :192] = np32(params["out_norm"])[:, None] * np32(params["head"])
    out["whead"] = whead

    consts = np.zeros((128, 2), np.float32)
    consts[:, :] = 1.0
    out["consts"] = consts
    out["allones"] = np.ones((128, 128), np.float32)
    oneblocks = np.zeros((128, 192), np.float32)
    oneblocks[:, 0:64] = 1.0
    oneblocks[:, 128:192] = 1.0
    out["oneblocks"] = oneblocks
    hsel = np.zeros((128, 2), np.float32)
    hsel[:64, 0] = 1.0
    hsel[64:, 1] = 1.0
    out["hsel"] = hsel
    biases = np.zeros((128, 2), np.float32)
    biases[:, 0] = EPS
    biases[:, 1] = HD * EPS
    out["biases"] = biases
    psw = np.zeros((128, 128), np.float32)
    for i in range(64):
        psw[2 * i, 2 * i + 1] = 1.0
        psw[2 * i + 1, 2 * i] = 1.0
    out["pswap"] = psw
    bones = np.zeros((128, 128), np.float32)
    bones[:64, :64] = 1.0
    bones[64:, 64:] = 1.0
    out["bones"] = bones
    return out


# ---------------------------------------------------------------- bass build

def _patch_act_tables():
    """Restrict the act-table chooser to {natural_log_exp_and_others, silu_and_others}
    so the greedy first-match picker stops thrashing between per-function sets.
    Indices are preserved (walrus maps set-id -> table by position)."""
    import concourse.hw_specs as hw_specs
    if getattr(hw_specs, "_act_tables_patched", False):
        return
    orig = hw_specs.get_activation_tables

    def patched(module_arch):
        tabs = orig(module_arch)
        keep = {"natural_log_exp_and_others", "silu_and_others"}
        return {k: (v if k in keep else set()) for k, v in tabs.items()}

    hw_specs.get_activation_tables = patched
    bacc.get_activation_tables = patched
    hw_specs._act_tables_patched = True


def build_nc(same_tables=True):
    _patch_act_tables()
    nc = bacc.Bacc()

    x0_d = nc.dram_tensor("x0", [256, NTOK], F32R, kind="ExternalInput")
    wpe_d = nc.dram_tensor("wpe", [256, 512], F32R, kind="ExternalInput")
    enorm_d = nc.dram_tensor("enorm", [128, DT], F32, kind="ExternalInput")
    whead_d = nc.dram_tensor("whead", [512, 256], F32R, kind="ExternalInput")
    consts_d = nc.dram_tensor("consts", [128, 2], F32R, kind="ExternalInput")
    allones_d = nc.dram_tensor("allones", [128, 128], F32R, kind="ExternalInput")
    oneblocks_d = nc.dram_tensor("oneblocks", [128, 192], F32R, kind="ExternalInput")
    pswap_d = nc.dram_tensor("pswap", [128, 128], F32R, kind="ExternalInput")
    bones_d = nc.dram_tensor("bones", [128, 128], F32R, kind="ExternalInput")
    biases_d = nc.dram_tensor("biases", [128, 2], F32, kind="ExternalInput")
    hsel_d = nc.dram_tensor("hsel", [128, 2], F32R, kind="ExternalInput")
    wqk_d, wv_d, wout_d, wgu_d, wdown_d, ropeq_d, ropek_d = [], [], [], [], [], [], []
    ntab = 1 if same_tables else NL
    for i in range(NL):
        wqk_d.append(nc.dram_tensor(f"wqk_{i}", [512, 1024], F32R, kind="ExternalInput"))
        wv_d.append(nc.dram_tensor(f"wv_{i}", [512, 512], F32R, kind="ExternalInput"))
        wout_d.append(nc.dram_tensor(f"wout_{i}", [512, 512], F32R, kind="ExternalInput"))
        wgu_d.append(nc.dram_tensor(f"wgu_{i}", [6, 512, 512], F32R, kind="ExternalInput"))
        wdown_d.append(nc.dram_tensor(f"wdown_{i}", [IHP, 512], F32R, kind="ExternalInput"))
    for i in range(ntab):
        ropeq_d.append(nc.dram_tensor(f"ropeq_{i}", [128, 1024], F32, kind="ExternalInput"))
        ropek_d.append(nc.dram_tensor(f"ropek_{i}", [128, 1024], F32, kind="ExternalInput"))
    out_d = nc.dram_tensor("out_tok", [NTOK, PD], F32, kind="ExternalOutput")

    with tile.TileContext(nc) as tc:
        with (
            tc.tile_pool(name="persist", bufs=1) as pp,
            tc.tile_pool(name="wqkp", bufs=1) as wqkp,
            tc.tile_pool(name="wvp", bufs=1) as wvp,
            tc.tile_pool(name="wop", bufs=1) as wop,
            tc.tile_pool(name="wgp", bufs=2) as wgp,
            tc.tile_pool(name="wdp", bufs=1) as wdp,
            tc.tile_pool(name="tabp", bufs=1) as tabp,
            tc.tile_pool(name="hp", bufs=2) as hp,
            tc.tile_pool(name="scr", bufs=2) as scr,
            tc.tile_pool(name="atp", bufs=1) as atp,
            tc.tile_pool(name="expp", bufs=3) as expp,
            tc.tile_pool(name="ps", bufs=2, space="PSUM") as ps,
            tc.tile_pool(name="psa", bufs=1, space="PSUM") as psa,
        ):
            x_all = pp.tile([128, DT, NTOK], F32)
            consts = pp.tile([128, 2], F32R)
            allones = pp.tile([128, 128], F32R)
            oneblocks = pp.tile([128, 192], F32R)
            nc.sync.dma_start(out=allones, in_=allones_d[:, :])
            nc.sync.dma_start(out=oneblocks, in_=oneblocks_d[:, :])
            nc.scalar.add_instruction(mybir.InstLoadActFuncSet(
                name=nc.get_next_instruction_name(), act_func_set_id=6, ins=[], outs=[]))
            pswap = pp.tile([128, 128], F32R)
            bones = pp.tile([128, 128], F32R)
            nc.sync.dma_start(out=consts, in_=consts_d[:, :])
            nc.sync.dma_start(out=pswap, in_=pswap_d[:, :])
            nc.sync.dma_start(out=bones, in_=bones_d[:, :])
            v_pad = pp.tile([128, 4, 4, 192], F32R)
            nc.vector.memset(bass.AP(tensor=v_pad.tensor, offset=v_pad.offset + 64,
                                     ap=[v_pad.ap[0], [192, 16], [1, 64]]).bitcast(F32), 0.0)
            biases = pp.tile([128, 2], F32)
            nc.sync.dma_start(out=biases, in_=biases_d[:, :])
            hsel = pp.tile([128, 2], F32R)
            nc.sync.dma_start(out=hsel, in_=hsel_d[:, :])
            eps_b = biases[:, 0:1]
            eps64_b = biases[:, 1:2]

            def load_tables(i):
                tabs = tabp.tile([128, 4, 512], F32, tag="tabs")  # cq sq ck sk
                nc.sync.dma_start(out=tabs[:, 0:2, :], in_=ropeq_d[i].rearrange("p (a m) -> p a m", a=2))
                nc.sync.dma_start(out=tabs[:, 2:4, :], in_=ropek_d[i].rearrange("p (a m) -> p a m", a=2))
                return tabs

            tabs0 = load_tables(0) if same_tables else None

            # h chunk [128, DT, 512] = x[:, :, cs] * rms_inv (stats pre-broadcast
            # via all-ones matmul: every output row = column sum)
            def make_inv(c):
                cs = slice(512 * c, 512 * (c + 1))
                ssqbc = ps.tile([128, 512], F32, tag="mm")
                for t in range(DT):
                    xsq = scr.tile([128, 512], F32R, tag="sq", bufs=2)
                    nc.gpsimd.tensor_mul(xsq, x_all[:, t, cs], x_all[:, t, cs])
                    nc.tensor.matmul(ssqbc, allones, xsq, start=(t == 0), stop=(t == DT - 1))
                # 1/sqrt(v) = exp(-0.5 * ln(v)); ln and exp share an ACT table set
                lnv = scr.tile([128, 512], F32, tag="lnv", bufs=1)
                nc.scalar.activation(lnv, ssqbc, AF.Ln, scale=1.0 / D, bias=eps_b)
                invbc = scr.tile([128, 512], F32, tag="invbc", bufs=4)
                nc.scalar.activation(invbc, lnv, AF.Exp, scale=-0.5)
                return invbc

            def make_h(c, invbc):
                cs = slice(512 * c, 512 * (c + 1))
                h = hp.tile([128, DT, 512], F32R, tag="h")
                for t in range(DT):
                    nc.gpsimd.tensor_mul(h[:, t, :], x_all[:, t, cs], invbc)
                return h

            # ---- patch embed
            wpe_sb = pp.tile([128, 2, 512], F32R)
            nc.sync.dma_start(out=wpe_sb, in_=wpe_d.rearrange("(t p) m -> p t m", p=128))
            enorm_sb = pp.tile([128, DT], F32)
            nc.sync.dma_start(out=enorm_sb, in_=enorm_d[:, :])
            for c in range(4):
                cs = slice(512 * c, 512 * (c + 1))
                x0c = scr.tile([128, 2, 512], F32R, tag="whead", bufs=1)
                nc.sync.dma_start(out=x0c, in_=x0_d.rearrange("(t p) m -> p t m", p=128)[:, :, cs])
                xe_ps = []
                for dout in range(DT):
                    pst = psa.tile([128, 512], F32, tag=f"acc{dout}", name=f"xe{dout}")
                    for din in range(2):
                        nc.tensor.matmul(pst, wpe_sb[:, din, 128 * dout:128 * (dout + 1)],
                                         x0c[:, din, :], start=(din == 0), stop=(din == 1))
                    xe_ps.append(pst)
                ssqbc = ps.tile([128, 512], F32, tag="mm")
                for t in range(DT):
                    xsq = scr.tile([128, 512], F32R, tag="sq", bufs=2)
                    nc.scalar.activation(xsq, xe_ps[t], AF.Square)
                    nc.tensor.matmul(ssqbc, allones, xsq, start=(t == 0), stop=(t == DT - 1))
                lnv = scr.tile([128, 512], F32, tag="lnv", bufs=1)
                nc.scalar.activation(lnv, ssqbc, AF.Ln, scale=1.0 / D, bias=eps_b)
                invbc = scr.tile([128, 512], F32, tag="invbc", bufs=4)
                nc.scalar.activation(invbc, lnv, AF.Exp, scale=-0.5)
                for t in range(DT):
                    tmp = scr.tile([128, 512], F32, tag="lnv", bufs=1)
                    nc.vector.tensor_mul(tmp, xe_ps[t], invbc)
                    nc.vector.tensor_scalar_mul(x_all[:, t, cs], tmp, enorm_sb[:, t:t + 1])

            # ---- transformer layers
            for li in range(NL):
                tabs = tabs0 if same_tables else load_tables(li)
                wqk = wqkp.tile([128, DT, 1024], F32R, tag="wqk")
                nc.sync.dma_start(out=wqk, in_=wqk_d[li].rearrange("(t p) m -> p t m", p=128))
                wv = wvp.tile([128, DT, 512], F32R, tag="wv")
                nc.sync.dma_start(out=wv, in_=wv_d[li].rearrange("(t p) m -> p t m", p=128))
                wout = wop.tile([128, DT, 512], F32R, tag="wout")
                nc.sync.dma_start(out=wout, in_=wout_d[li].rearrange("(t p) m -> p t m", p=128))
                wdown = wdp.tile([128, 6, 512], F32R, tag="wdown")
                nc.sync.dma_start(out=wdown, in_=wdown_d[li].rearrange("(t p) m -> p t m", p=128))

                for b in range(BL):
                    h = make_h(b, make_inv(b))
                    # v token-major, padded layout: per (lt, f): [v_h0 | 0(64) | v_h1]
                    for lt in range(4):
                        vps = ps.tile([128, 512], F32, tag="mm")
                        for d in range(DT):
                            nc.tensor.matmul(vps, h[:, d, 128 * lt:128 * (lt + 1)],
                                             wv[:, d, :], start=(d == 0), stop=(d == DT - 1))
                        vdst = bass.AP(tensor=v_pad.tensor,
                                       offset=v_pad.offset + lt * 768,
                                       ap=[v_pad.ap[0], [192, 4], [128, 2], [1, 64]])
                        nc.scalar.copy(vdst, vps[:, :].rearrange("p (f a e) -> p f a e", f=4, a=2))

                    # invk accumulator [128(l), 4(ls), 8(h) x 2]
                    ikps = psa.tile([128, 4, NH], F32, tag="acc2")
                    invk_lh = atp.tile([128, 4, NH], F32, tag="invk_lh")

                    o_fm = atp.tile([128, DT, 512], F32R, tag="o_fm")
                    for f in range(DT):
                        # q, k raw (feature-major) for this feat tile
                        q_raw = scr.tile([128, 512], F32R, tag="q_raw", bufs=1)
                        k_raw = scr.tile([128, 512], F32R, tag="k_raw", bufs=1)
                        for which, dst in ((0, q_raw), (1, k_raw)):
                            qkps = ps.tile([128, 512], F32, tag="mm")
                            for d in range(DT):
                                nc.tensor.matmul(qkps, wqk[:, d, 512 * which + 128 * f:512 * which + 128 * (f + 1)],
                                                 h[:, d, :], start=(d == 0), stop=(d == DT - 1))
                            nc.scalar.copy(dst, qkps)

                        # invq pre-broadcast (includes 1/8 attn scale)
                        qsq = scr.tile([128, 512], F32R, tag="sq", bufs=2)
                        nc.gpsimd.tensor_mul(qsq, q_raw, q_raw)
                        sbc = ps.tile([128, 512], F32, tag="mm")
                        nc.tensor.matmul(sbc, bones, qsq, start=True, stop=True)
                        lnq = scr.tile([128, 512], F32, tag="lnv", bufs=1)
                        nc.scalar.activation(lnq, sbc, AF.Ln, bias=eps64_b)
                        invq_bc = scr.tile([128, 512], F32, tag="invq_bc")
                        nc.scalar.activation(invq_bc, lnq, AF.Exp, scale=-0.5)

                        # invk token-major (this tile's 2 heads)
                        ksq = scr.tile([128, 512], F32R, tag="sq2", bufs=1)
                        nc.gpsimd.tensor_mul(ksq, k_raw, k_raw)
                        for ls in range(4):
                            nc.tensor.matmul(
                                ikps[:, ls, 2 * f:2 * f + 2],
                                ksq[:, 128 * ls:128 * (ls + 1)],
                                hsel[:, 0:2],
                                start=True, stop=True)
                        lnk = scr.tile([128, 8], F32, tag="lnk", bufs=1)
                        nc.scalar.activation(lnk, ikps[:, :, 2 * f:2 * f + 2],
                                             AF.Ln, scale=1.0 / HD, bias=eps_b)
                        nc.scalar.activation(invk_lh[:, :, 2 * f:2 * f + 2], lnk, AF.Exp, scale=-0.5)

                        # rope
                        q_rope = scr.tile([128, 512], F32R, tag="q_rope", bufs=2)
                        k_rope = scr.tile([128, 512], F32R, tag="k_rope", bufs=2)
                        swq = ps.tile([128, 512], F32, tag="sps")
                        nc.tensor.matmul(swq, pswap, q_raw, start=True, stop=True)
                        qc = scr.tile([128, 512], F32, tag="qc", bufs=1)
                        nc.vector.tensor_mul(qc, q_raw, tabs[:, 0, :])
                        qs = scr.tile([128, 512], F32, tag="qs", bufs=1)
                        nc.vector.tensor_mul(qs, swq, tabs[:, 1, :])
                        nc.vector.tensor_add(qc, qc, qs)
                        nc.vector.tensor_mul(q_rope, qc, invq_bc)
                        swk = ps.tile([128, 512], F32, tag="sps")
                        nc.tensor.matmul(swk, pswap, k_raw, start=True, stop=True)
                        kc = scr.tile([128, 512], F32, tag="kc", bufs=1)
                        nc.gpsimd.tensor_mul(kc, k_raw, tabs[:, 2, :])
                        ks = scr.tile([128, 512], F32, tag="ks", bufs=1)
                        nc.vector.tensor_mul(ks, swk, tabs[:, 3, :])
                        nc.gpsimd.tensor_add(k_rope, kc, ks)

                        # attention for this tile's two heads; head hh lands on
                        # output rows [64*hh, 64*hh+64) via padded stationaries
                        ops = psa.tile([128, 512], F32, tag="acc0" if f % 2 == 0 else "acc3")
                        rps = psa.tile([128, 512], F32, tag="acc1")
                        for t in range(4):
                            for hh in range(2):
                                hidx = 2 * f + hh
                                hsl = slice(64 * hh, 64 * (hh + 1))
                                qcols = slice(128 * t, 512)
                                # fp32r needs moving dim >= 256 for full rate:
                                # widen reads to 256 with zeroed filler columns
                                mcols = slice(min(128 * t, 256), 512)
                                sps = ps.tile([128, 512], F32, tag="sps")
                                nc.tensor.matmul(sps[:, mcols],
                                                 k_rope[hsl, 128 * t:128 * (t + 1)],
                                                 q_rope[hsl, mcols],
                                                 start=True, stop=True)
                                expt = expp.tile([128, 512], F32R, tag="expt")
                                nc.scalar.activation(expt[:, qcols], sps[:, qcols], AF.Exp,
                                                     scale=invk_lh[:, t, hidx:hidx + 1])
                                nc.gpsimd.memset(
                                    expt[64:128, 128 * t:128 * t + 64].bitcast(F32), 0.0)
                                if t == 3:
                                    nc.gpsimd.memset(expt[:, 256:384].bitcast(F32), 0.0)
                                first = (hh == 0 and t == 0)
                                last = (hh == 1 and t == 3)
                                nc.tensor.matmul(rps[:, mcols],
                                                 oneblocks[:, 64 * hh:64 * hh + 128],
                                                 expt[:, mcols],
                                                 start=first, stop=last)
                                nc.tensor.matmul(ops[:, mcols],
                                                 v_pad[:, t, f, 64 * hh:64 * hh + 128],
                                                 expt[:, mcols],
                                                 start=first, stop=last)
                        rrec = scr.tile([128, 512], F32, tag="rrec", bufs=1)
                        nc.vector.reciprocal(rrec, rps)
                        nc.vector.tensor_mul(o_fm[:, f, :], ops, rrec)

                    # out proj + residual
                    bs = slice(512 * b, 512 * (b + 1))
                    for dout in range(DT):
                        xps = ps.tile([128, 512], F32, tag="sps")
                        for d in range(DT):
                            nc.tensor.matmul(xps, wout[:, d, 128 * dout:128 * (dout + 1)],
                                             o_fm[:, d, :], start=(d == 0), stop=(d == DT - 1))
                        nc.vector.tensor_add(x_all[:, dout, bs], xps, x_all[:, dout, bs])

                # ---- mlp (stats first so ln/exp cluster before silus)
                mlp_invs = [make_inv(c) for c in range(4)]
                for c in range(4):
                    cs = slice(512 * c, 512 * (c + 1))
                    h = make_h(c, mlp_invs[c])
                    dps = [psa.tile([128, 512], F32, tag=f"acc{t}", name=f"dps{t}") for t in range(DT)]
                    for f in range(6):
                        wgu = wgp.tile([128, DT, 512], F32R, tag="wgu")
                        nc.sync.dma_start(out=wgu, in_=wgu_d[li][f].rearrange("(t p) m -> p t m", p=128))

                        def gu_mm(j):
                            pps = ps.tile([128, 512], F32, tag="sps", name=f"gu{j}")
                            for d in range(DT):
                                nc.tensor.matmul(pps, wgu[:, d, 128 * j:128 * (j + 1)],
                                                 h[:, d, :], start=(d == 0), stop=(d == DT - 1))
                            return pps
                        # order: g1, silu, u1, mul, g2, silu, u2, mul -> <=2 live psums
                        p0 = gu_mm(0)
                        s1 = scr.tile([128, 512], F32, tag="s1", bufs=1)
                        nc.scalar.activation(s1, p0, AF.Silu)
                        p2 = gu_mm(2)
                        t1 = scr.tile([128, 512], F32, tag="t1", bufs=1)
                        nc.vector.tensor_mul(t1, s1, p2)
                        p1 = gu_mm(1)
                        s2 = scr.tile([128, 512], F32, tag="s2", bufs=1)
                        nc.scalar.activation(s2, p1, AF.Silu)
                        p3 = gu_mm(3)
                        t2 = scr.tile([128, 512], F32, tag="t2", bufs=1)
                        nc.vector.tensor_mul(t2, s2, p3)
                        ug = scr.tile([128, 512], F32R, tag="ug", bufs=1)
                        nc.gpsimd.tensor_add(ug, t1, t2)
                        for dout in range(DT):
                            nc.tensor.matmul(dps[dout], wdown[:, f, 128 * dout:128 * (dout + 1)],
                                             ug, start=(f == 0), stop=(f == 5))
                    for dout in range(DT):
                        nc.vector.tensor_add(x_all[:, dout, cs], dps[dout], x_all[:, dout, cs])

            # ---- final norm + head
            whead_sb = scr.tile([128, DT, 256], F32R, tag="whead", bufs=1)
            nc.sync.dma_start(out=whead_sb, in_=whead_d.rearrange("(t p) m -> p t m", p=128))
            head_invs = [make_inv(c) for c in range(4)]
            for c in range(4):
                h = make_h(c, head_invs[c])
                for lt in range(4):
                    hps = ps.tile([128, 256], F32, tag="mm")
                    for d in range(DT):
                        nc.tensor.matmul(hps, h[:, d, 128 * lt:128 * (lt + 1)],
                                         whead_sb[:, d, :], start=(d == 0), stop=(d == DT - 1))
                    osb = scr.tile([128, PD], F32, tag="osb")
                    nc.scalar.copy(osb, hps[:, 0:PD])
                    nc.sync.dma_start(out=out_d[512 * c + 128 * lt:512 * c + 128 * (lt + 1), :], in_=osb)

    nc.finalize()
    return nc


# ---------------------------------------------------------------- entry

_CACHE = {}


def kernel(frames, params):
    frames = np.asarray(frames, dtype=np.float32)
    prep = host_prep(params)
    same_tables = prep["same_tables"]

    if "nc" not in _CACHE:
        _CACHE["nc"] = build_nc(same_tables=same_tables)
    nc = _CACHE["nc"]

    x0 = _patchify(frames)
    shared = {k: v for k, v in prep.items() if isinstance(v, np.ndarray)}
    in_maps = []
    for core in range(NCORES):
        m = dict(shared)
        xb = x0[core * BL:(core + 1) * BL].reshape(NTOK, PD)
        x0f = np.zeros((256, NTOK), np.float32)
        x0f[:PD] = xb.T
        m["x0"] = x0f
        in_maps.append(m)

    res = run_bass_kernel_spmd(nc, in_maps, core_ids=list(range(NCORES)))
    outs = []
    for core in range(NCORES):
        tok = res.results[core]["out_tok"].reshape(BL, L, PD)
        outs.append(_unpatchify(tok))
    return np.concatenate(outs, axis=0)


# revision 39
# speedup vs baseline: 1.1307x; 1.0000x over previous
"""Trainium2 Bass kernel for nn_ARVideoPatchTransformer_80436147519663.

Distribution: data-parallel over batch (B=32 -> 4 samples/core x 8 cores),
no collectives. Each core runs the full transformer on its shard.

On-chip dataflow (per core):
- Residual x is feature-major [D(4x128 partition-tiles), 2048 tokens], fp32.
- All matmuls run in float32r (full PE rate at N>=256, ~14-bit mantissa),
  accumulating fp32 in PSUM.
- QKV emits q,k feature-major (scores need hd on partitions) and v
  token-major (AV matmul wants l_k on partitions). No PE transposes anywhere.
- RoPE pair-swap = PE permutation matmul; rope mul/add on DVE (q) and
  GPSIMD (k) to balance engines.
- Softmax in scoresT orientation [l_k, l_q]: block-causal mask realized by
  restricting matmul column ranges + one 64x64 corner memset per tile;
  per-head q-rms (with 1/sqrt(HD) folded) is produced pre-broadcast by a
  block-ones stationary matmul; k-rms lands token-major via stationary-k^2
  matmuls and is folded into the exp() activation scale. Softmax denominators
  come from a 64-col ones matmul (pre-broadcast), one reciprocal per 2 heads.
"""
import math
import numpy as np

import concourse.bass as bass
from concourse import bacc
import concourse.mybir as mybir
import concourse.tile as tile
from concourse.bass_utils import run_bass_kernel_spmd

F32 = mybir.dt.float32
F32R = mybir.dt.float32r
AF = mybir.ActivationFunctionType
ALU = mybir.AluOpType

B, T, C, RES, P = 32, 8, 3, 64, 8
NP = (RES // P) ** 2          # 64
L = T * NP                    # 512
PD = C * P * P                # 192
D, NH, NL = 512, 8, 8
HD = D // NH                  # 64
RD = HD // 2                  # 32
INNER = 1364
IH = INNER // 2               # 682
IHP = 768                     # padded half (6*128)
MAXT = T + 1
EPS = 1e-6
NCORES = 8
BL = B // NCORES              # 4
NTOK = BL * L                 # 2048
DT = D // 128                 # 4


# ---------------------------------------------------------------- host prep

def _sin_cos(rotary_dim, max_len, base=10000.0):
    inv = 1.0 / (base ** (np.arange(0, rotary_dim, 2, dtype=np.float32) / rotary_dim))
    ang = np.outer(np.arange(max_len, dtype=np.float32), inv)
    ang = np.repeat(ang, 2, axis=-1)
    return np.cos(ang).astype(np.float32), np.sin(ang).astype(np.float32)


def _build_rope_tables(q_scale, k_scale):
    """[128(2 heads' feats), 512(l)] C and S tables with scale folded."""
    t_cos, t_sin = _sin_cos(RD, MAXT)
    s_cos, s_sin = _sin_cos(RD, NP)
    l = np.arange(L)
    cos_full = np.concatenate([t_cos[l // NP], s_cos[l % NP]], axis=1)  # [512, 64]
    sin_full = np.concatenate([t_sin[l // NP], s_sin[l % NP]], axis=1)

    def fold(s):
        s = np.asarray(s, np.float32)
        Cm = cos_full * s[None, :]
        Sm = np.empty_like(sin_full)
        Sm[:, 0::2] = -sin_full[:, 0::2] * s[None, 1::2]
        Sm[:, 1::2] = sin_full[:, 1::2] * s[None, 0::2]
        return np.tile(Cm.T, (2, 1)).copy(), np.tile(Sm.T, (2, 1)).copy()  # [128, 512]

    return fold(q_scale), fold(k_scale)


def _patchify(frames):
    b, t, c, hh, ww = frames.shape
    h = hh // P
    x = frames.reshape(b * t, c, h, P, h, P)
    x = x.transpose(0, 2, 4, 1, 3, 5)
    return x.reshape(b, t * h * h, c * P * P)


def _unpatchify(tokens):
    b, l, _ = tokens.shape
    h = RES // P
    t = l // (h * h)
    x = tokens.reshape(b * t, h, h, C, P, P)
    x = x.transpose(0, 3, 1, 4, 2, 5)
    return x.reshape(b, t, C, h * P, h * P)


def host_prep(params):
    def np32(a):
        return np.ascontiguousarray(np.asarray(a, dtype=np.float32))

    out = {}
    wpe = np.zeros((256, 512), np.float32)
    wpe[:192] = np32(params["patch_embed"])
    out["wpe"] = wpe
    out["enorm"] = np32(params["embed_norm"]).reshape(DT, 128).T.copy()  # [128, 4]

    qs0 = np32(params["layers"][0]["q_scale"])
    ks0 = np32(params["layers"][0]["k_scale"])
    same_tables = all(
        np.array_equal(np32(lp["q_scale"]), qs0) and np.array_equal(np32(lp["k_scale"]), ks0)
        for lp in params["layers"])
    out["same_tables"] = same_tables

    for i, lp in enumerate(params["layers"]):
        wqkv = np32(lp["qkv"]) * np32(lp["norm1"])[:, None]
        out[f"wqk_{i}"] = np.ascontiguousarray(wqkv[:, :1024])
        out[f"wv_{i}"] = np.ascontiguousarray(wqkv[:, 1024:])
        out[f"wout_{i}"] = np32(lp["out"])
        gate = np32(lp["gate"]) * np32(lp["norm2"])[:, None]
        up = np32(lp["up"]) * np32(lp["norm2"])[:, None]
        wgu = np.zeros((6, 512, 512), np.float32)
        for f in range(6):
            lo, hi = f * 128, min(f * 128 + 128, IH)
            if hi > lo:
                for j, src in enumerate((gate[:, :IH], gate[:, IH:], up[:, :IH], up[:, IH:])):
                    wgu[f, :, j * 128:j * 128 + hi - lo] = src[:, lo:hi]
        out[f"wgu_{i}"] = wgu
        wdown = np.zeros((IHP, 512), np.float32)
        wdown[:IH] = np32(lp["down"])
        out[f"wdown_{i}"] = wdown
        if i == 0 or not same_tables:
            (cq, sq), (ck, sk) = _build_rope_tables(lp["q_scale"], lp["k_scale"])
            out[f"ropeq_{i}"] = np.concatenate([cq, sq], axis=1)   # [128, 1024]
            out[f"ropek_{i}"] = np.concatenate([ck, sk], axis=1)

    whead = np.zeros((512, 256), np.float32)
    whead[:, :192] = np32(params["out_norm"])[:, None] * np32(params["head"])
    out["whead"] = whead

    consts = np.zeros((128, 2), np.float32)
    consts[:, :] = 1.0
    out["consts"] = consts
    out["allones"] = np.ones((128, 128), np.float32)
    oneblocks = np.zeros((128, 192), np.float32)
    oneblocks[:, 0:64] = 1.0
    oneblocks[:, 128:192] = 1.0
    out["oneblocks"] = oneblocks
    hsel = np.zeros((128, 2), np.float32)
    hsel[:64, 0] = 1.0
    hsel[64:, 1] = 1.0
    out["hsel"] = hsel
    biases = np.zeros((128, 2), np.float32)
    biases[:, 0] = EPS
    biases[:, 1] = HD * EPS
    out["biases"] = biases
    psw = np.zeros((128, 128), np.float32)
    for i in range(64):
        psw[2 * i, 2 * i + 1] = 1.0
        psw[2 * i + 1, 2 * i] = 1.0
    out["pswap"] = psw
    bones = np.zeros((128, 128), np.float32)
    bones[:64, :64] = 1.0
    bones[64:, 64:] = 1.0
    out["bones"] = bones
    return out


# ---------------------------------------------------------------- bass build

def _patch_act_tables():
    """Restrict the act-table chooser to {natural_log_exp_and_others, silu_and_others}
    so the greedy first-match picker stops thrashing between per-function sets.
    Indices are preserved (walrus maps set-id -> table by position)."""
    import concourse.hw_specs as hw_specs
    if getattr(hw_specs, "_act_tables_patched", False):
        return
    orig = hw_specs.get_activation_tables

    def patched(module_arch):
        tabs = orig(module_arch)
        keep = {"natural_log_exp_and_others", "silu_and_others"}
        return {k: (v if k in keep else set()) for k, v in tabs.items()}

    hw_specs.get_activation_tables = patched
    bacc.get_activation_tables = patched
    hw_specs._act_tables_patched = True


def build_nc(same_tables=True):
    _patch_act_tables()
    nc = bacc.Bacc()

    x0_d = nc.dram_tensor("x0", [256, NTOK], F32R, kind="ExternalInput")
    wpe_d = nc.dram_tensor("wpe", [256, 512], F32R, kind="ExternalInput")
    enorm_d = nc.dram_tensor("enorm", [128, DT], F32, kind="ExternalInput")
    whead_d = nc.dram_tensor("whead", [512, 256], F32R, kind="ExternalInput")
    consts_d = nc.dram_tensor("consts", [128, 2], F32R, kind="ExternalInput")
    allones_d = nc.dram_tensor("allones", [128, 128], F32R, kind="ExternalInput")
    oneblocks_d = nc.dram_tensor("oneblocks", [128, 192], F32R, kind="ExternalInput")
    pswap_d = nc.dram_tensor("pswap", [128, 128], F32R, kind="ExternalInput")
    bones_d = nc.dram_tensor("bones", [128, 128], F32R, kind="ExternalInput")
    biases_d = nc.dram_tensor("biases", [128, 2], F32, kind="ExternalInput")
    hsel_d = nc.dram_tensor("hsel", [128, 2], F32R, kind="ExternalInput")
    wqk_d, wv_d, wout_d, wgu_d, wdown_d, ropeq_d, ropek_d = [], [], [], [], [], [], []
    ntab = 1 if same_tables else NL
    for i in range(NL):
        wqk_d.append(nc.dram_tensor(f"wqk_{i}", [512, 1024], F32R, kind="ExternalInput"))
        wv_d.append(nc.dram_tensor(f"wv_{i}", [512, 512], F32R, kind="ExternalInput"))
        wout_d.append(nc.dram_tensor(f"wout_{i}", [512, 512], F32R, kind="ExternalInput"))
        wgu_d.append(nc.dram_tensor(f"wgu_{i}", [6, 512, 512], F32R, kind="ExternalInput"))
        wdown_d.append(nc.dram_tensor(f"wdown_{i}", [IHP, 512], F32R, kind="ExternalInput"))
    for i in range(ntab):
        ropeq_d.append(nc.dram_tensor(f"ropeq_{i}", [128, 1024], F32, kind="ExternalInput"))
        ropek_d.append(nc.dram_tensor(f"ropek_{i}", [128, 1024], F32, kind="ExternalInput"))
    out_d = nc.dram_tensor("out_tok", [NTOK, PD], F32, kind="ExternalOutput")

    with tile.TileContext(nc) as tc:
        with (
            tc.tile_pool(name="persist", bufs=1) as pp,
            tc.tile_pool(name="wqkp", bufs=1) as wqkp,
            tc.tile_pool(name="wvp", bufs=1) as wvp,
            tc.tile_pool(name="wop", bufs=1) as wop,
            tc.tile_pool(name="wgp", bufs=2) as wgp,
            tc.tile_pool(name="wdp", bufs=1) as wdp,
            tc.tile_pool(name="tabp", bufs=1) as tabp,
            tc.tile_pool(name="hp", bufs=2) as hp,
            tc.tile_pool(name="scr", bufs=2) as scr,
            tc.tile_pool(name="atp", bufs=1) as atp,
            tc.tile_pool(name="expp", bufs=3) as expp,
            tc.tile_pool(name="ps", bufs=2, space="PSUM") as ps,
            tc.tile_pool(name="psa", bufs=1, space="PSUM") as psa,
        ):
            x_all = pp.tile([128, DT, NTOK], F32)
            consts = pp.tile([128, 2], F32R)
            allones = pp.tile([128, 128], F32R)
            oneblocks = pp.tile([128, 192], F32R)
            nc.sync.dma_start(out=allones, in_=allones_d[:, :])
            nc.sync.dma_start(out=oneblocks, in_=oneblocks_d[:, :])
            nc.scalar.add_instruction(mybir.InstLoadActFuncSet(
                name=nc.get_next_instruction_name(), act_func_set_id=6, ins=[], outs=[]))
            pswap = pp.tile([128, 128], F32R)
            bones = pp.tile([128, 128], F32R)
            nc.sync.dma_start(out=consts, in_=consts_d[:, :])
            nc.sync.dma_start(out=pswap, in_=pswap_d[:, :])
            nc.sync.dma_start(out=bones, in_=bones_d[:, :])
            v_pad = pp.tile([128, 4, 4, 192], F32R)
            nc.vector.memset(bass.AP(tensor=v_pad.tensor, offset=v_pad.offset + 64,
                                     ap=[v_pad.ap[0], [192, 16], [1, 64]]).bitcast(F32), 0.0)
            biases = pp.tile([128, 2], F32)
            nc.sync.dma_start(out=biases, in_=biases_d[:, :])
            hsel = pp.tile([128, 2], F32R)
            nc.sync.dma_start(out=hsel, in_=hsel_d[:, :])
            eps_b = biases[:, 0:1]
            eps64_b = biases[:, 1:2]

            def load_tables(i):
                tabs = tabp.tile([128, 4, 512], F32, tag="tabs")  # cq sq ck sk
                nc.sync.dma_start(out=tabs[:, 0:2, :], in_=ropeq_d[i].rearrange("p (a m) -> p a m", a=2))
                nc.sync.dma_start(out=tabs[:, 2:4, :], in_=ropek_d[i].rearrange("p (a m) -> p a m", a=2))
                return tabs

            tabs0 = load_tables(0) if same_tables else None

            # h chunk [128, DT, 512] = x[:, :, cs] * rms_inv (stats pre-broadcast
            # via all-ones matmul: every output row = column sum)
            def make_inv(c):
                cs = slice(512 * c, 512 * (c + 1))
                ssqbc = ps.tile([128, 512], F32, tag="mm")
                for t in range(DT):
                    xsq = scr.tile([128, 512], F32R, tag="sq", bufs=2)
                    nc.gpsimd.tensor_mul(xsq, x_all[:, t, cs], x_all[:, t, cs])
                    nc.tensor.matmul(ssqbc, allones, xsq, start=(t == 0), stop=(t == DT - 1))
                # 1/sqrt(v) = exp(-0.5 * ln(v)); ln and exp share an ACT table set
                lnv = scr.tile([128, 512], F32, tag="lnv", bufs=1)
                nc.scalar.activation(lnv, ssqbc, AF.Ln, scale=1.0 / D, bias=eps_b)
                invbc = scr.tile([128, 512], F32, tag="invbc", bufs=4)
                nc.scalar.activation(invbc, lnv, AF.Exp, scale=-0.5)
                return invbc

            def make_h(c, invbc):
                cs = slice(512 * c, 512 * (c + 1))
                h = hp.tile([128, DT, 512], F32R, tag="h")
                for t in range(DT):
                    nc.gpsimd.tensor_mul(h[:, t, :], x_all[:, t, cs], invbc)
                return h

            # ---- patch embed
            wpe_sb = pp.tile([128, 2, 512], F32R)
            nc.sync.dma_start(out=wpe_sb, in_=wpe_d.rearrange("(t p) m -> p t m", p=128))
            enorm_sb = pp.tile([128, DT], F32)
            nc.sync.dma_start(out=enorm_sb, in_=enorm_d[:, :])
            for c in range(4):
                cs = slice(512 * c, 512 * (c + 1))
                x0c = scr.tile([128, 2, 512], F32R, tag="whead", bufs=1)
                nc.sync.dma_start(out=x0c, in_=x0_d.rearrange("(t p) m -> p t m", p=128)[:, :, cs])
                xe_ps = []
                for dout in range(DT):
                    pst = psa.tile([128, 512], F32, tag=f"acc{dout}", name=f"xe{dout}")
                    for din in range(2):
                        nc.tensor.matmul(pst, wpe_sb[:, din, 128 * dout:128 * (dout + 1)],
                                         x0c[:, din, :], start=(din == 0), stop=(din == 1))
                    xe_ps.append(pst)
                ssqbc = ps.tile([128, 512], F32, tag="mm")
                for t in range(DT):
                    xsq = scr.tile([128, 512], F32R, tag="sq", bufs=2)
                    nc.scalar.activation(xsq, xe_ps[t], AF.Square)
                    nc.tensor.matmul(ssqbc, allones, xsq, start=(t == 0), stop=(t == DT - 1))
                lnv = scr.tile([128, 512], F32, tag="lnv", bufs=1)
                nc.scalar.activation(lnv, ssqbc, AF.Ln, scale=1.0 / D, bias=eps_b)
                invbc = scr.tile([128, 512], F32, tag="invbc", bufs=4)
                nc.scalar.activation(invbc, lnv, AF.Exp, scale=-0.5)
                for t in range(DT):
                    tmp = scr.tile([128, 512], F32, tag="lnv", bufs=1)
                    nc.vector.tensor_mul(tmp, xe_ps[t], invbc)
                    nc.vector.tensor_scalar_mul(x_all[:, t, cs], tmp, enorm_sb[:, t:t + 1])

            # ---- transformer layers
            for li in range(NL):
                tabs = tabs0 if same_tables else load_tables(li)
                wqk = wqkp.tile([128, DT, 1024], F32R, tag="wqk")
                wqk_src = wqk_d[li].rearrange("(t p) m -> p t m", p=128)
                for cch in range(4):
                    nc.sync.dma_start(out=wqk[:, :, 256 * cch:256 * (cch + 1)],
                                      in_=wqk_src[:, :, 256 * cch:256 * (cch + 1)])
                wv = wvp.tile([128, DT, 512], F32R, tag="wv")
                wv_src = wv_d[li].rearrange("(t p) m -> p t m", p=128)
                for cch in range(2):
                    nc.sync.dma_start(out=wv[:, :, 256 * cch:256 * (cch + 1)],
                                      in_=wv_src[:, :, 256 * cch:256 * (cch + 1)])
                wout = wop.tile([128, DT, 512], F32R, tag="wout")
                nc.sync.dma_start(out=wout, in_=wout_d[li].rearrange("(t p) m -> p t m", p=128))
                wdown = wdp.tile([128, 6, 512], F32R, tag="wdown")
                nc.sync.dma_start(out=wdown, in_=wdown_d[li].rearrange("(t p) m -> p t m", p=128))

                # prefetch first two MLP weight blocks so the attention->MLP
                # transition doesn't wait on their DMAs
                wgu_pre = []
                for f in range(2):
                    wgu = wgp.tile([128, DT, 512], F32R, tag="wgu", name=f"wgupre{f}")
                    nc.sync.dma_start(out=wgu, in_=wgu_d[li][f].rearrange("(t p) m -> p t m", p=128))
                    wgu_pre.append(wgu)

                for b in range(BL):
                    h = make_h(b, make_inv(b))
                    # v token-major, padded layout: per (lt, f): [v_h0 | 0(64) | v_h1]
                    for lt in range(4):
                        vps = ps.tile([128, 512], F32, tag="mm")
                        for d in range(DT):
                            nc.tensor.matmul(vps, h[:, d, 128 * lt:128 * (lt + 1)],
                                             wv[:, d, :], start=(d == 0), stop=(d == DT - 1))
                        vdst = bass.AP(tensor=v_pad.tensor,
                                       offset=v_pad.offset + lt * 768,
                                       ap=[v_pad.ap[0], [192, 4], [128, 2], [1, 64]])
                        nc.scalar.copy(vdst, vps[:, :].rearrange("p (f a e) -> p f a e", f=4, a=2))

                    # invk accumulator [128(l), 4(ls), 8(h) x 2]
                    ikps = psa.tile([128, 4, NH], F32, tag="acc2")
                    invk_lh = atp.tile([128, 4, NH], F32, tag="invk_lh")

                    o_fm = atp.tile([128, DT, 512], F32R, tag="o_fm")
                    for f in range(DT):
                        # q, k raw (feature-major) for this feat tile
                        q_raw = scr.tile([128, 512], F32R, tag="q_raw", bufs=1)
                        k_raw = scr.tile([128, 512], F32R, tag="k_raw", bufs=1)
                        for which, dst in ((0, q_raw), (1, k_raw)):
                            qkps = ps.tile([128, 512], F32, tag="mm")
                            for d in range(DT):
                                nc.tensor.matmul(qkps, wqk[:, d, 512 * which + 128 * f:512 * which + 128 * (f + 1)],
                                                 h[:, d, :], start=(d == 0), stop=(d == DT - 1))
                            nc.scalar.copy(dst, qkps)

                        # invq pre-broadcast (includes 1/8 attn scale)
                        qsq = scr.tile([128, 512], F32R, tag="sq", bufs=2)
                        nc.gpsimd.tensor_mul(qsq, q_raw, q_raw)
                        sbc = ps.tile([128, 512], F32, tag="mm")
                        nc.tensor.matmul(sbc, bones, qsq, start=True, stop=True)
                        lnq = scr.tile([128, 512], F32, tag="lnv", bufs=1)
                        nc.scalar.activation(lnq, sbc, AF.Ln, bias=eps64_b)
                        invq_bc = scr.tile([128, 512], F32, tag="invq_bc")
                        nc.scalar.activation(invq_bc, lnq, AF.Exp, scale=-0.5)

                        # invk token-major (this tile's 2 heads)
                        ksq = scr.tile([128, 512], F32R, tag="sq2", bufs=1)
                        nc.gpsimd.tensor_mul(ksq, k_raw, k_raw)
                        for ls in range(4):
                            nc.tensor.matmul(
                                ikps[:, ls, 2 * f:2 * f + 2],
                                ksq[:, 128 * ls:128 * (ls + 1)],
                                hsel[:, 0:2],
                                start=True, stop=True)
                        lnk = scr.tile([128, 8], F32, tag="lnk", bufs=1)
                        nc.scalar.activation(lnk, ikps[:, :, 2 * f:2 * f + 2],
                                             AF.Ln, scale=1.0 / HD, bias=eps_b)
                        nc.scalar.activation(invk_lh[:, :, 2 * f:2 * f + 2], lnk, AF.Exp, scale=-0.5)

                        # rope
                        q_rope = scr.tile([128, 512], F32R, tag="q_rope", bufs=2)
                        k_rope = scr.tile([128, 512], F32R, tag="k_rope", bufs=2)
                        swq = ps.tile([128, 512], F32, tag="sps")
                        nc.tensor.matmul(swq, pswap, q_raw, start=True, stop=True)
                        qc = scr.tile([128, 512], F32, tag="qc", bufs=1)
                        nc.vector.tensor_mul(qc, q_raw, tabs[:, 0, :])
                        qs = scr.tile([128, 512], F32, tag="qs", bufs=1)
                        nc.vector.tensor_mul(qs, swq, tabs[:, 1, :])
                        nc.vector.tensor_add(qc, qc, qs)
                        nc.vector.tensor_mul(q_rope, qc, invq_bc)
                        swk = ps.tile([128, 512], F32, tag="sps")
                        nc.tensor.matmul(swk, pswap, k_raw, start=True, stop=True)
                        kc = scr.tile([128, 512], F32, tag="kc", bufs=1)
                        nc.gpsimd.tensor_mul(kc, k_raw, tabs[:, 2, :])
                        ks = scr.tile([128, 512], F32, tag="ks", bufs=1)
                        nc.vector.tensor_mul(ks, swk, tabs[:, 3, :])
                        nc.gpsimd.tensor_add(k_rope, kc, ks)

                        # attention for this tile's two heads; head hh lands on
                        # output rows [64*hh, 64*hh+64) via padded stationaries
                        ops = psa.tile([128, 512], F32, tag="acc0" if f % 2 == 0 else "acc3")
                        rps = psa.tile([128, 512], F32, tag="acc1")
                        for t in range(4):
                            for hh in range(2):
                                hidx = 2 * f + hh
                                hsl = slice(64 * hh, 64 * (hh + 1))
                                qcols = slice(128 * t, 512)
                                # fp32r needs moving dim >= 256 for full rate:
                                # widen reads to 256 with zeroed filler columns
                                mcols = slice(min(128 * t, 256), 512)
                                sps = ps.tile([128, 512], F32, tag="sps")
                                nc.tensor.matmul(sps[:, mcols],
                                                 k_rope[hsl, 128 * t:128 * (t + 1)],
                                                 q_rope[hsl, mcols],
                                                 start=True, stop=True)
                                expt = expp.tile([128, 512], F32R, tag="expt")
                                nc.scalar.activation(expt[:, qcols], sps[:, qcols], AF.Exp,
                                                     scale=invk_lh[:, t, hidx:hidx + 1])
                                nc.gpsimd.memset(
                                    expt[64:128, 128 * t:128 * t + 64].bitcast(F32), 0.0)
                                if t == 3:
                                    nc.gpsimd.memset(expt[:, 256:384].bitcast(F32), 0.0)
                                first = (hh == 0 and t == 0)
                                last = (hh == 1 and t == 3)
                                nc.tensor.matmul(rps[:, mcols],
                                                 oneblocks[:, 64 * hh:64 * hh + 128],
                                                 expt[:, mcols],
                                                 start=first, stop=last)
                                nc.tensor.matmul(ops[:, mcols],
                                                 v_pad[:, t, f, 64 * hh:64 * hh + 128],
                                                 expt[:, mcols],
                                                 start=first, stop=last)
                        rrec = scr.tile([128, 512], F32, tag="rrec", bufs=1)
                        nc.vector.reciprocal(rrec, rps)
                        nc.vector.tensor_mul(o_fm[:, f, :], ops, rrec)

                    # out proj + residual
                    bs = slice(512 * b, 512 * (b + 1))
                    for dout in range(DT):
                        xps = ps.tile([128, 512], F32, tag="sps")
                        for d in range(DT):
                            nc.tensor.matmul(xps, wout[:, d, 128 * dout:128 * (dout + 1)],
                                             o_fm[:, d, :], start=(d == 0), stop=(d == DT - 1))
                        nc.vector.tensor_add(x_all[:, dout, bs], xps, x_all[:, dout, bs])

                # ---- mlp (stats first so ln/exp cluster before silus)
                mlp_invs = [make_inv(c) for c in range(4)]
                for c in range(4):
                    cs = slice(512 * c, 512 * (c + 1))
                    h = make_h(c, mlp_invs[c])
                    dps = [psa.tile([128, 512], F32, tag=f"acc{t}", name=f"dps{t}") for t in range(DT)]
                    for f in range(6):
                        if c == 0 and f < 2:
                            wgu = wgu_pre[f]
                        else:
                            wgu = wgp.tile([128, DT, 512], F32R, tag="wgu")
                            nc.sync.dma_start(out=wgu, in_=wgu_d[li][f].rearrange("(t p) m -> p t m", p=128))

                        def gu_mm(j):
                            pps = ps.tile([128, 512], F32, tag="sps", name=f"gu{j}")
                            for d in range(DT):
                                nc.tensor.matmul(pps, wgu[:, d, 128 * j:128 * (j + 1)],
                                                 h[:, d, :], start=(d == 0), stop=(d == DT - 1))
                            return pps
                        # order: g1, silu, u1, mul, g2, silu, u2, mul -> <=2 live psums
                        p0 = gu_mm(0)
                        s1 = scr.tile([128, 512], F32, tag="s1", bufs=1)
                        nc.scalar.activation(s1, p0, AF.Silu)
                        p2 = gu_mm(2)
                        t1 = scr.tile([128, 512], F32, tag="t1", bufs=1)
                        nc.vector.tensor_mul(t1, s1, p2)
                        p1 = gu_mm(1)
                        s2 = scr.tile([128, 512], F32, tag="s2", bufs=1)
                        nc.scalar.activation(s2, p1, AF.Silu)
                        p3 = gu_mm(3)
                        t2 = scr.tile([128, 512], F32, tag="t2", bufs=1)
                        nc.vector.tensor_mul(t2, s2, p3)
                        ug = scr.tile([128, 512], F32R, tag="ug", bufs=1)
                        nc.gpsimd.tensor_add(ug, t1, t2)
                        for dout in range(DT):
                            nc.tensor.matmul(dps[dout], wdown[:, f, 128 * dout:128 * (dout + 1)],
                                             ug, start=(f == 0), stop=(f == 5))
                    for dout in range(DT):
                        nc.vector.tensor_add(x_all[:, dout, cs], dps[dout], x_all[:, dout, cs])

            # ---- final norm + head
            whead_sb = scr.tile([128, DT, 256], F32R, tag="whead", bufs=1)
            nc.sync.dma_start(out=whead_sb, in_=whead_d.rearrange("(t p) m -> p t m", p=128))
            head_invs = [make_inv(c) for c in range(4)]
            for c in range(4):
                h = make_h(c, head_invs[c])
                for lt in range(4):
                    hps = ps.tile([128, 256], F32, tag="mm")
                    for d in range(DT):
                        nc.tensor.matmul(hps, h[:, d, 128 * lt:128 * (lt + 1)],
                                         whead_sb[:, d, :], start=(d == 0), stop=(d == DT - 1))
                    osb = scr.tile([128, PD], F32, tag="osb")
                    nc.scalar.copy(osb, hps[:, 0:PD])
                    nc.sync.dma_start(out=out_d[512 * c + 128 * lt:512 * c + 128 * (lt + 1), :], in_=osb)

    nc.finalize()
    return nc


# ---------------------------------------------------------------- entry

_CACHE = {}


def kernel(frames, params):
    frames = np.asarray(frames, dtype=np.float32)
    prep = host_prep(params)
    same_tables = prep["same_tables"]

    if "nc" not in _CACHE:
        _CACHE["nc"] = build_nc(same_tables=same_tables)
    nc = _CACHE["nc"]

    x0 = _patchify(frames)
    shared = {k: v for k, v in prep.items() if isinstance(v, np.ndarray)}
    in_maps = []
    for core in range(NCORES):
        m = dict(shared)
        xb = x0[core * BL:(core + 1) * BL].reshape(NTOK, PD)
        x0f = np.zeros((256, NTOK), np.float32)
        x0f[:PD] = xb.T
        m["x0"] = x0f
        in_maps.append(m)

    res = run_bass_kernel_spmd(nc, in_maps, core_ids=list(range(NCORES)))
    outs = []
    for core in range(NCORES):
        tok = res.results[core]["out_tok"].reshape(BL, L, PD)
        outs.append(_unpatchify(tok))
    return np.concatenate(outs, axis=0)
